# revision 1
# baseline (speedup 1.0000x reference)
"""Trainium2 Bass kernel for CausalTrajectoryPrediction (batched per-branch MLPs).

Math (per branch n of N=512, H=256, M=64):
    x_off = x with element n zeroed ; x_diag = only element n kept
    h1 = relu(W1a[n] @ x_off)            # [H]
    r1 = relu(W1b[n] @ h1)               # [M]
    r2 = relu(W2b[n] @ relu(W2a[n] @ x_diag + b2a[n]) + b2b[n])   # [1]
    h4 = relu(W4a[n] @ [r1; r2] + b4a[n])                          # [H]
    pred = relu(W4b[n] @ h4 + b4b[n])                              # [2]
    out[n] = pred[0] + noise[n] * pred[1]

Device strategy (8 cores, 64 branches each, expert-parallel):
  * W1a[n] @ x_off == W1a[n] @ x - W1a[n,:,n]*x[n]; the shared matvec is done
    on-device (weights stationary, x moving), the diagonal correction is a
    tiny host-side gather shipped as a [128,128] tile.
  * The self-excite path (r2) only touches W2a's diagonal -> computed on host
    (512 branches x ~8 flops) and folded into an effective stage-4 bias:
    bias4_eff = b4a + W4a[:,:,64] * r2.  Stage 4 then contracts over m=0..63.
  * All weights are pre-transposed on host into [K-partition, free] layouts so
    each per-branch matvec is a single self-loading f32 matmul with the
    activation vector as the moving operand; outputs land on PSUM partitions
    in exactly the layout the next stage consumes.
  * DMAs all issue on the sync (HWDGE/SP) ring -> FIFO in program order:
    32MB W1a stream first (paces stage-1 PE), then stage-2/4 weights arrive
    just-in-time for the tail stages.
"""

import numpy as np

import concourse.bacc as bacc
import concourse.bass as bass
import concourse.mybir as mybir
import concourse.tile as tile
from concourse.bass_utils import run_bass_kernel_spmd

F32 = mybir.dt.float32
NCORES = 8
N, H, M = 512, 256, 64
J = N // NCORES  # 64 branches per core

_CACHE = {}


def _build_nc():
    if "nc" in _CACHE:
        return _CACHE["nc"]

    nc = bacc.Bacc(
        "TRN2", target_bir_lowering=False, debug=False, enable_asserts=False,
        num_devices=NCORES,
    )

    # --- DRAM I/O (per-core shapes) ---
    # w1t[i, j*256+h]              = W1a[g, h, i]           (g = 64*core + j)
    # w2t[hl, j*128+hc*64+m]       = W1b[g, m, hc*128+hl]
    # w4t[64*(j%2)+m, (j//2)*256+h]= W4a[g, h, m]   (m<64; col 64 folded in bias)
    # w5t[hl, j*4+hc*2+o]          = W4b[g, o, hc*128+hl]
    # aux1 = [xcols(4) | corr(128) | bias4(128)]   -> [128, 260]
    #   xcols[p, ic] = x[128*ic+p]
    #   corr[p, 2j+hh]  = W1a[g, hh*128+p, g] * x[g]
    #   bias4[p, 2j+hh] = b4a[g, hh*128+p] + W4a[g, hh*128+p, 64]*r2_host[g]
    # aux2 = [b4bt(64) | noise2(64) | ones(1)]     -> [2, 129]
    w1t = nc.dram_tensor("w1t", [512, J * H], F32, kind="ExternalInput").ap()
    w2t = nc.dram_tensor("w2t", [128, J * 2 * 64], F32, kind="ExternalInput").ap()
    w4t = nc.dram_tensor("w4t", [128, (J // 2) * H], F32, kind="ExternalInput").ap()
    w5t = nc.dram_tensor("w5t", [128, J * 4], F32, kind="ExternalInput").ap()
    aux1 = nc.dram_tensor("aux1", [128, 260], F32, kind="ExternalInput").ap()
    aux2 = nc.dram_tensor("aux2", [2, 129], F32, kind="ExternalInput").ap()
    out = nc.dram_tensor("out", [1, J], F32, kind="ExternalOutput").ap()

    NWCHUNK = 8  # stage-1 free-dim chunks of 2048 cols (1 MiB per DMA)

    with tile.TileContext(nc) as tc:
        with (
            tc.tile_pool(name="stream", bufs=8) as sp,
            tc.tile_pool(name="res", bufs=1) as rp,
            tc.tile_pool(name="psum", bufs=1, space=bass.MemorySpace.PSUM) as pp,
        ):
            # small resident tensors first (tiny; off the critical DMA path)
            aux1_t = rp.tile([128, 260], F32)
            nc.sync.dma_start(aux1_t[:, :], aux1)
            aux2_t = rp.tile([2, 129], F32)
            nc.sync.dma_start(aux2_t[:, :], aux2)
            w5s = rp.tile([128, J * 4], F32)
            nc.sync.dma_start(w5s[:, :], w5t)

            xc = aux1_t[:, 0:4]
            corrt = aux1_t[:, 4:132]
            bias4t = aux1_t[:, 132:260]
            b4bt = aux2_t[:, 0:64]
            noise2 = aux2_t[:, 64:128]
            ones2 = aux2_t[:, 128:129]

            w2s = rp.tile([128, J * 2 * 64], F32)
            w4s = rp.tile([128, (J // 2) * H], F32)

            y1 = pp.tile([128, 128], F32)

            # ---- stage 1: y1[j,h] = sum_i W1a[g,h,i] x[i] ----
            # 128 psum columns t = 2j+hh ; 4 accumulating matmuls each (i-chunks)
            for u in range(NWCHUNK):
                tiles_u = []
                for ic in range(4):
                    wt = sp.tile([128, 2048], F32, tag="w1s")
                    nc.sync.dma_start(
                        wt[:, :], w1t[128 * ic : 128 * (ic + 1), 2048 * u : 2048 * (u + 1)]
                    )
                    tiles_u.append(wt)
                for tt in range(16):
                    t = 16 * u + tt
                    for ic in range(4):
                        nc.tensor.matmul(
                            y1[:, t : t + 1],
                            tiles_u[ic][:, 128 * tt : 128 * (tt + 1)],
                            xc[:, ic : ic + 1],
                            start=(ic == 0),
                            stop=(ic == 3),
                        )

            # stage-2/4 weights arrive after the w1t stream (sync ring is FIFO)
            for k in range(4):
                nc.sync.dma_start(
                    w2s[:, 2048 * k : 2048 * (k + 1)], w2t[:, 2048 * k : 2048 * (k + 1)]
                )
            for k in range(4):
                nc.sync.dma_start(
                    w4s[:, 2048 * k : 2048 * (k + 1)], w4t[:, 2048 * k : 2048 * (k + 1)]
                )

            # h1 = relu(y1 - corr)
            h1sb = rp.tile([128, 128], F32)
            nc.vector.tensor_sub(h1sb[:, :], y1[:, :], corrt)
            nc.vector.tensor_scalar_max(h1sb[:, :], h1sb[:, :], 0.0)

            # ---- stage 2: r1[j,m] = relu(sum_h W1b[g,m,h] h1[j,h]) ----
            # even j -> psum rows 0..63, odd j -> rows 64..127 (col-group tiling)
            psum2 = pp.tile([128, J], F32)
            nc.vector.memset(psum2[:, :], 0.0)
            for j in range(J):
                off = 64 * (j % 2)
                for hc in range(2):
                    nc.tensor.matmul(
                        psum2[off : off + 64, j : j + 1],
                        w2s[:, j * 128 + hc * 64 : j * 128 + hc * 64 + 64],
                        h1sb[:, 2 * j + hc : 2 * j + hc + 1],
                        start=(hc == 0),
                        stop=(hc == 1),
                    )
            r1cols = rp.tile([128, J], F32)
            nc.vector.tensor_scalar_max(r1cols[:, :], psum2[:, :], 0.0)

            # ---- stage 4: h4[j,h] = relu(sum_m W4a[g,h,m] r1[j,m] + bias4eff) ----
            psum4 = pp.tile([128, 128], F32)
            for j in range(J):
                off = 64 * (j % 2)
                u_ = j // 2
                for hc in range(2):
                    nc.tensor.matmul(
                        psum4[:, 2 * j + hc : 2 * j + hc + 1],
                        w4s[off : off + 64, u_ * 256 + hc * 128 : u_ * 256 + hc * 128 + 128],
                        r1cols[off : off + 64, j : j + 1],
                        start=True,
                        stop=True,
                    )
            h4cols = rp.tile([128, 128], F32)
            nc.vector.tensor_add(h4cols[:, :], psum4[:, :], bias4t)
            nc.vector.tensor_scalar_max(h4cols[:, :], h4cols[:, :], 0.0)

            # ---- stage 5: pred[o,j] = relu(sum_h W4b[g,o,h] h4[j,h] + b4b) ----
            psum5 = pp.tile([2, J], F32)
            for j in range(J):
                for hc in range(2):
                    nc.tensor.matmul(
                        psum5[0:2, j : j + 1],
                        w5s[:, j * 4 + hc * 2 : j * 4 + hc * 2 + 2],
                        h4cols[:, 2 * j + hc : 2 * j + hc + 1],
                        start=(hc == 0),
                        stop=(hc == 1),
                    )
            pred = rp.tile([2, J], F32)
            nc.vector.tensor_add(pred[:, :], psum5[:, :], b4bt)
            nc.vector.tensor_scalar_max(pred[:, :], pred[:, :], 0.0)

            # out = pred[0] + noise*pred[1]  (partition-sum via K=2 matmul)
            scaled = rp.tile([2, J], F32)
            nc.vector.tensor_mul(scaled[:, :], pred[:, :], noise2)
            psum6 = pp.tile([1, J], F32)
            nc.tensor.matmul(psum6[0:1, :], ones2, scaled[:, :], start=True, stop=True)
            yrow = rp.tile([1, J], F32)
            nc.vector.tensor_copy(yrow[0:1, :], psum6[0:1, :])
            nc.sync.dma_start(out, yrow[0:1, :])

    nc.compile()
    _CACHE["nc"] = nc
    return nc


def prep_core_inputs(inputs, c):
    """Host-side shard + layout prep for core c. inputs are np float32 arrays."""
    x = inputs["x"][0]  # [512]
    gi = np.arange(J * c, J * (c + 1))
    xg = x[gi]
    jj = np.arange(J)

    W1a_c = inputs["W1a"][gi]  # [64, 256, 512]
    w1t = np.ascontiguousarray(W1a_c.transpose(2, 0, 1).reshape(512, J * H))

    # self-excite path entirely on host (tiny), folded into stage-4 bias
    dW2 = inputs["W2a"][gi, :, gi]  # [64, 2]
    h2 = np.maximum(dW2 * xg[:, None] + inputs["b2a"][gi], 0.0)
    r2 = np.maximum((inputs["W2b"][gi, 0] * h2).sum(-1) + inputs["b2b"][gi, 0], 0.0)

    dW1 = W1a_c[jj, :, gi]  # [64, 256] : W1a[g, h, g]
    corr_jh = dW1 * xg[:, None]
    corrt = corr_jh.reshape(J, 2, 128).transpose(2, 0, 1).reshape(128, 128)

    W4a_c = inputs["W4a"][gi]  # [64, 256, 65]
    bias4_jh = inputs["b4a"][gi] + W4a_c[:, :, 64] * r2[:, None]
    bias4t = bias4_jh.reshape(J, 2, 128).transpose(2, 0, 1).reshape(128, 128)

    xcols = x.reshape(4, 128).T  # [128, 4]
    aux1 = np.ascontiguousarray(
        np.concatenate([xcols, corrt, bias4t], axis=1), dtype=np.float32
    )

    b4bt = inputs["b4b"][gi].T  # [2, 64]
    noise2 = np.stack([np.ones(J, np.float32), inputs["noise"][gi]])
    ones2 = np.ones((2, 1), np.float32)
    aux2 = np.ascontiguousarray(
        np.concatenate([b4bt, noise2, ones2], axis=1), dtype=np.float32
    )

    W1b_c = inputs["W1b"][gi]  # [64, 64, 256]
    w2t = np.ascontiguousarray(
        W1b_c.reshape(J, 64, 2, 128).transpose(3, 0, 2, 1).reshape(128, J * 2 * 64)
    )

    # stage 4: branch pairs stacked on partitions (even j -> rows 0..63)
    W4m = W4a_c[:, :, 0:64]  # [j, h, m]
    T4 = W4m.reshape(J // 2, 2, H, 64)  # [u, par, h, m]
    w4t = np.ascontiguousarray(T4.transpose(1, 3, 0, 2).reshape(128, (J // 2) * H))

    W4b_c = inputs["W4b"][gi]  # [64, 2, 256]
    w5t = np.ascontiguousarray(
        W4b_c.reshape(J, 2, 2, 128).transpose(3, 0, 2, 1).reshape(128, J * 4)
    )

    return {
        "w1t": w1t, "w2t": w2t, "w4t": w4t, "w5t": w5t,
        "aux1": aux1, "aux2": aux2,
    }


def run(inputs, trace=False, **kw):
    inputs = {k: np.asarray(v, dtype=np.float32) for k, v in inputs.items()}
    nc = _build_nc()
    in_maps = [prep_core_inputs(inputs, c) for c in range(NCORES)]
    res = run_bass_kernel_spmd(
        nc, in_maps, core_ids=list(range(NCORES)), trace=trace, **kw
    )
    out = np.concatenate([res.results[c]["out"] for c in range(NCORES)], axis=1)
    return out.astype(np.float32), res


def kernel(**inputs):
    out, _ = run(inputs)
    return out


# revision 2
# speedup vs baseline: 3.3948x; 3.3948x over previous
"""Trainium2 Bass kernel for CausalTrajectoryPrediction (batched per-branch MLPs).

Math (per branch n of N=512, H=256, M=64):
    x_off = x with element n zeroed ; x_diag = only element n kept
    h1 = relu(W1a[n] @ x_off)            # [H]
    r1 = relu(W1b[n] @ h1)               # [M]
    r2 = relu(W2b[n] @ relu(W2a[n] @ x_diag + b2a[n]) + b2b[n])   # [1]
    h4 = relu(W4a[n] @ [r1; r2] + b4a[n])                          # [H]
    pred = relu(W4b[n] @ h4 + b4b[n])                              # [2]
    out[n] = pred[0] + noise[n] * pred[1]

Device strategy (8 cores, 64 branches each, expert-parallel):
  * W1a[n] @ x_off == W1a[n] @ x - W1a[n,:,n]*x[n]; the shared matvec is done
    on-device (weights stationary, x moving), the diagonal correction is a
    tiny host-side gather shipped as a [128,128] f32 tile (computed from the
    bf16-rounded operands so the subtraction cancels exactly).
  * The self-excite path (r2) only touches W2a's diagonal -> computed on host
    (512 branches x ~8 flops) and folded into an effective stage-4 bias:
    bias4_eff = b4a + W4a[:,:,64] * r2.  Stage 4 then contracts over m=0..63.
  * Weights + moving activations in bf16 (PSUM accumulation f32, all bias /
    correction math f32): f32 matmuls lower to 2x LDWEIGHTS+MATMUL passes on
    TRN2, and bf16 stationaries get fast-weight-load; bf16 also halves the
    32MB weight stream.
  * All weights are pre-transposed on host into [K-partition, free] layouts so
    each per-branch matvec is a single matmul with the activation vector as
    the moving operand; outputs land on PSUM partitions in exactly the layout
    the next stage consumes.
  * DMAs all issue on the sync (HWDGE/SP) ring -> FIFO in program order:
    the W1a stream first (paces stage-1 PE), then stage-2/4 weights arrive
    just-in-time for the tail stages.
"""

import ml_dtypes
import numpy as np

import concourse.bacc as bacc
import concourse.bass as bass
import concourse.mybir as mybir
import concourse.tile as tile
from concourse.bass_utils import run_bass_kernel_spmd

F32 = mybir.dt.float32
BF16 = mybir.dt.bfloat16
NP_BF16 = ml_dtypes.bfloat16
NCORES = 8
N, H, M = 512, 256, 64
J = N // NCORES  # 64 branches per core

_CACHE = {}


def _build_nc():
    if "nc" in _CACHE:
        return _CACHE["nc"]

    nc = bacc.Bacc(
        "TRN2", target_bir_lowering=False, debug=False, enable_asserts=False,
        num_devices=NCORES,
    )

    # --- DRAM I/O (per-core shapes) ---
    # w1t[i, j*256+h]              = W1a[g, h, i]           (g = 64*core + j)
    # w2t[hl, j*128+hc*64+m]       = W1b[g, m, hc*128+hl]
    # w4t[64*(j%2)+m, (j//2)*256+h]= W4a[g, h, m]   (m<64; col 64 folded in bias)
    # w5t[hl, j*4+hc*2+o]          = W4b[g, o, hc*128+hl]
    # xbf[p, ic] = x[128*ic+p]                     (bf16 moving operand)
    # aux1 = [corr(128) | bias4(128)]   -> [128, 256] f32
    #   corr[p, 2j+hh]  = bf16(W1a[g, hh*128+p, g]) * bf16(x[g])
    #   bias4[p, 2j+hh] = b4a[g, hh*128+p] + W4a[g, hh*128+p, 64]*r2_host[g]
    # aux2 = [b4bt(64) | noise2(64) | ones(1)]     -> [2, 129] f32
    w1t = nc.dram_tensor("w1t", [512, J * H], BF16, kind="ExternalInput").ap()
    w2t = nc.dram_tensor("w2t", [128, J * 2 * 64], BF16, kind="ExternalInput").ap()
    w4t = nc.dram_tensor("w4t", [128, (J // 2) * H], BF16, kind="ExternalInput").ap()
    w5t = nc.dram_tensor("w5t", [128, J * 4], BF16, kind="ExternalInput").ap()
    xbf = nc.dram_tensor("xbf", [128, 4], BF16, kind="ExternalInput").ap()
    aux1 = nc.dram_tensor("aux1", [128, 256], F32, kind="ExternalInput").ap()
    aux2 = nc.dram_tensor("aux2", [2, 129], F32, kind="ExternalInput").ap()
    out = nc.dram_tensor("out", [1, J], F32, kind="ExternalOutput").ap()

    NWCHUNK = 4  # stage-1 free-dim chunks of 4096 cols (1 MiB bf16 per DMA)

    with tile.TileContext(nc) as tc:
        with (
            tc.tile_pool(name="stream", bufs=8) as sp,
            tc.tile_pool(name="res", bufs=1) as rp,
            tc.tile_pool(name="psum", bufs=1, space=bass.MemorySpace.PSUM) as pp,
        ):
            # small resident tensors first (tiny; off the critical DMA path)
            xbf_t = rp.tile([128, 4], BF16)
            nc.sync.dma_start(xbf_t[:, :], xbf)
            aux1_t = rp.tile([128, 256], F32)
            nc.sync.dma_start(aux1_t[:, :], aux1)
            aux2_t = rp.tile([2, 129], F32)
            nc.sync.dma_start(aux2_t[:, :], aux2)
            w5s = rp.tile([128, J * 4], BF16)
            nc.sync.dma_start(w5s[:, :], w5t)

            corrt = aux1_t[:, 0:128]
            bias4t = aux1_t[:, 128:256]
            b4bt = aux2_t[:, 0:64]
            noise2 = aux2_t[:, 64:128]
            ones2 = aux2_t[:, 128:129]

            w2s = rp.tile([128, J * 2 * 64], BF16)
            w4s = rp.tile([128, (J // 2) * H], BF16)

            y1 = pp.tile([128, 128], F32)

            # ---- stage 1: y1[j,h] = sum_i W1a[g,h,i] x[i] ----
            # 128 psum columns t = 2j+hh ; 4 accumulating matmuls each (i-chunks)
            for u in range(NWCHUNK):
                tiles_u = []
                for ic in range(4):
                    wt = sp.tile([128, 4096], BF16, tag="w1s")
                    nc.sync.dma_start(
                        wt[:, :], w1t[128 * ic : 128 * (ic + 1), 4096 * u : 4096 * (u + 1)]
                    )
                    tiles_u.append(wt)
                for tt in range(32):
                    t = 32 * u + tt
                    for ic in range(4):
                        nc.tensor.matmul(
                            y1[:, t : t + 1],
                            tiles_u[ic][:, 128 * tt : 128 * (tt + 1)],
                            xbf_t[:, ic : ic + 1],
                            start=(ic == 0),
                            stop=(ic == 3),
                        )

            # stage-2/4 weights arrive after the w1t stream (sync ring is FIFO)
            for k in range(2):
                nc.sync.dma_start(
                    w2s[:, 4096 * k : 4096 * (k + 1)], w2t[:, 4096 * k : 4096 * (k + 1)]
                )
            for k in range(2):
                nc.sync.dma_start(
                    w4s[:, 4096 * k : 4096 * (k + 1)], w4t[:, 4096 * k : 4096 * (k + 1)]
                )

            # h1 = relu(y1 - corr)  (bf16 for the next stage's moving operand)
            h1f = rp.tile([128, 128], F32)
            nc.vector.tensor_sub(h1f[:, :], y1[:, :], corrt)
            h1sb = rp.tile([128, 128], BF16)
            nc.vector.tensor_scalar_max(h1sb[:, :], h1f[:, :], 0.0)

            # ---- stage 2: r1[j,m] = relu(sum_h W1b[g,m,h] h1[j,h]) ----
            # even j -> psum rows 0..63, odd j -> rows 64..127 (col-group tiling)
            psum2 = pp.tile([128, J], F32)
            nc.vector.memset(psum2[:, :], 0.0)
            for j in range(J):
                off = 64 * (j % 2)
                for hc in range(2):
                    nc.tensor.matmul(
                        psum2[off : off + 64, j : j + 1],
                        w2s[:, j * 128 + hc * 64 : j * 128 + hc * 64 + 64],
                        h1sb[:, 2 * j + hc : 2 * j + hc + 1],
                        start=(hc == 0),
                        stop=(hc == 1),
                    )
            r1cols = rp.tile([128, J], BF16)
            nc.vector.tensor_scalar_max(r1cols[:, :], psum2[:, :], 0.0)

            # ---- stage 4: h4[j,h] = relu(sum_m W4a[g,h,m] r1[j,m] + bias4eff) ----
            psum4 = pp.tile([128, 128], F32)
            for j in range(J):
                off = 64 * (j % 2)
                u_ = j // 2
                for hc in range(2):
                    nc.tensor.matmul(
                        psum4[:, 2 * j + hc : 2 * j + hc + 1],
                        w4s[off : off + 64, u_ * 256 + hc * 128 : u_ * 256 + hc * 128 + 128],
                        r1cols[off : off + 64, j : j + 1],
                        start=True,
                        stop=True,
                    )
            h4f = rp.tile([128, 128], F32)
            nc.vector.tensor_add(h4f[:, :], psum4[:, :], bias4t)
            h4cols = rp.tile([128, 128], BF16)
            nc.vector.tensor_scalar_max(h4cols[:, :], h4f[:, :], 0.0)

            # ---- stage 5: pred[o,j] = relu(sum_h W4b[g,o,h] h4[j,h] + b4b) ----
            psum5 = pp.tile([2, J], F32)
            for j in range(J):
                for hc in range(2):
                    nc.tensor.matmul(
                        psum5[0:2, j : j + 1],
                        w5s[:, j * 4 + hc * 2 : j * 4 + hc * 2 + 2],
                        h4cols[:, 2 * j + hc : 2 * j + hc + 1],
                        start=(hc == 0),
                        stop=(hc == 1),
                    )
            pred = rp.tile([2, J], F32)
            nc.vector.tensor_add(pred[:, :], psum5[:, :], b4bt)
            nc.vector.tensor_scalar_max(pred[:, :], pred[:, :], 0.0)

            # out = pred[0] + noise*pred[1]  (partition-sum via K=2 f32 matmul)
            scaled = rp.tile([2, J], F32)
            nc.vector.tensor_mul(scaled[:, :], pred[:, :], noise2)
            psum6 = pp.tile([1, J], F32)
            nc.tensor.matmul(psum6[0:1, :], ones2, scaled[:, :], start=True, stop=True)
            yrow = rp.tile([1, J], F32)
            nc.vector.tensor_copy(yrow[0:1, :], psum6[0:1, :])
            nc.sync.dma_start(out, yrow[0:1, :])

    nc.compile()
    _CACHE["nc"] = nc
    return nc


def _bf(a):
    return np.ascontiguousarray(a.astype(NP_BF16))


def prep_core_inputs(inputs, c):
    """Host-side shard + layout prep for core c. inputs are np float32 arrays."""
    x = inputs["x"][0]  # [512]
    gi = np.arange(J * c, J * (c + 1))
    xg = x[gi]
    jj = np.arange(J)

    W1a_c = inputs["W1a"][gi]  # [64, 256, 512]
    w1t = _bf(W1a_c.transpose(2, 0, 1).reshape(512, J * H))

    # self-excite path entirely on host (tiny), folded into stage-4 bias
    dW2 = inputs["W2a"][gi, :, gi]  # [64, 2]
    h2 = np.maximum(dW2 * xg[:, None] + inputs["b2a"][gi], 0.0)
    r2 = np.maximum((inputs["W2b"][gi, 0] * h2).sum(-1) + inputs["b2b"][gi, 0], 0.0)

    # correction computed from the bf16-rounded operands (exact cancellation
    # of the diagonal term the device's bf16 matmul actually added)
    dW1 = W1a_c[jj, :, gi].astype(NP_BF16).astype(np.float32)  # [64, 256]
    xg_bf = xg.astype(NP_BF16).astype(np.float32)
    corr_jh = dW1 * xg_bf[:, None]
    corrt = corr_jh.reshape(J, 2, 128).transpose(2, 0, 1).reshape(128, 128)

    W4a_c = inputs["W4a"][gi]  # [64, 256, 65]
    bias4_jh = inputs["b4a"][gi] + W4a_c[:, :, 64] * r2[:, None]
    bias4t = bias4_jh.reshape(J, 2, 128).transpose(2, 0, 1).reshape(128, 128)

    aux1 = np.ascontiguousarray(
        np.concatenate([corrt, bias4t], axis=1), dtype=np.float32
    )
    xbf = _bf(x.reshape(4, 128).T)  # [128, 4]

    b4bt = inputs["b4b"][gi].T  # [2, 64]
    noise2 = np.stack([np.ones(J, np.float32), inputs["noise"][gi]])
    ones2 = np.ones((2, 1), np.float32)
    aux2 = np.ascontiguousarray(
        np.concatenate([b4bt, noise2, ones2], axis=1), dtype=np.float32
    )

    W1b_c = inputs["W1b"][gi]  # [64, 64, 256]
    w2t = _bf(
        W1b_c.reshape(J, 64, 2, 128).transpose(3, 0, 2, 1).reshape(128, J * 2 * 64)
    )

    # stage 4: branch pairs stacked on partitions (even j -> rows 0..63)
    W4m = W4a_c[:, :, 0:64]  # [j, h, m]
    T4 = W4m.reshape(J // 2, 2, H, 64)  # [u, par, h, m]
    w4t = _bf(T4.transpose(1, 3, 0, 2).reshape(128, (J // 2) * H))

    W4b_c = inputs["W4b"][gi]  # [64, 2, 256]
    w5t = _bf(W4b_c.reshape(J, 2, 2, 128).transpose(3, 0, 2, 1).reshape(128, J * 4))

    return {
        "w1t": w1t, "w2t": w2t, "w4t": w4t, "w5t": w5t,
        "xbf": xbf, "aux1": aux1, "aux2": aux2,
    }


def run(inputs, trace=False, **kw):
    inputs = {k: np.asarray(v, dtype=np.float32) for k, v in inputs.items()}
    nc = _build_nc()
    in_maps = [prep_core_inputs(inputs, c) for c in range(NCORES)]
    res = run_bass_kernel_spmd(
        nc, in_maps, core_ids=list(range(NCORES)), trace=trace, **kw
    )
    out = np.concatenate([res.results[c]["out"] for c in range(NCORES)], axis=1)
    return out.astype(np.float32), res


def kernel(**inputs):
    out, _ = run(inputs)
    return out


# revision 3
# speedup vs baseline: 3.6165x; 1.0653x over previous
"""Trainium2 Bass kernel for CausalTrajectoryPrediction (batched per-branch MLPs).

Math (per branch n of N=512, H=256, M=64):
    x_off = x with element n zeroed ; x_diag = only element n kept
    h1 = relu(W1a[n] @ x_off)            # [H]
    r1 = relu(W1b[n] @ h1)               # [M]
    r2 = relu(W2b[n] @ relu(W2a[n] @ x_diag + b2a[n]) + b2b[n])   # [1]
    h4 = relu(W4a[n] @ [r1; r2] + b4a[n])                          # [H]
    pred = relu(W4b[n] @ h4 + b4b[n])                              # [2]
    out[n] = pred[0] + noise[n] * pred[1]

Device strategy (8 cores, 64 branches each, expert-parallel):
  * W1a[n] @ x_off == W1a[n] @ x - W1a[n,:,n]*x[n]; the shared matvec is done
    on-device (weights stationary, x moving), the diagonal correction is a
    tiny host-side gather shipped as a [128,128] f32 tile (computed from the
    bf16-rounded operands so the subtraction cancels exactly).
  * The self-excite path (r2) only touches W2a's diagonal -> computed on host
    (512 branches x ~8 flops) and folded into an effective stage-4 bias:
    bias4_eff = b4a + W4a[:,:,64] * r2.  Stage 4 then contracts over m=0..63.
  * Weights + moving activations in bf16 (PSUM accumulation f32, all bias /
    correction math f32): f32 matmuls lower to 2x LDWEIGHTS+MATMUL passes on
    TRN2, and bf16 stationaries get fast-weight-load; bf16 also halves the
    32MB weight stream.
  * All weights are pre-transposed on host into [K-partition, free] layouts so
    each per-branch matvec is a single matmul with the activation vector as
    the moving operand; outputs land on PSUM partitions in exactly the layout
    the next stage consumes.
  * DMAs all issue on the sync (HWDGE/SP) ring -> FIFO in program order:
    the W1a stream first (paces stage-1 PE), then stage-2/4 weights arrive
    just-in-time for the tail stages.
"""

import ml_dtypes
import numpy as np

import concourse.bacc as bacc
import concourse.bass as bass
import concourse.mybir as mybir
import concourse.tile as tile
from concourse.bass_utils import run_bass_kernel_spmd

F32 = mybir.dt.float32
BF16 = mybir.dt.float16
NP_BF16 = np.float16
NCORES = 8
N, H, M = 512, 256, 64
J = N // NCORES  # 64 branches per core

_CACHE = {}


def _build_nc():
    if "nc" in _CACHE:
        return _CACHE["nc"]

    nc = bacc.Bacc(
        "TRN2", target_bir_lowering=False, debug=False, enable_asserts=False,
        num_devices=NCORES,
    )

    # --- DRAM I/O (per-core shapes) ---
    # w1t[i, j*256+h]              = W1a[g, h, i]           (g = 64*core + j)
    # w2t[hl, j*128+hc*64+m]       = W1b[g, m, hc*128+hl]
    # w4t[64*(j%2)+m, (j//2)*256+h]= W4a[g, h, m]   (m<64; col 64 folded in bias)
    # w5t[hl, j*4+hc*2+o]          = W4b[g, o, hc*128+hl]
    # xbf[p, ic] = x[128*ic+p]                     (bf16 moving operand)
    # aux1 = [corr(128) | bias4(128)]   -> [128, 256] f32
    #   corr[p, 2j+hh]  = bf16(W1a[g, hh*128+p, g]) * bf16(x[g])
    #   bias4[p, 2j+hh] = b4a[g, hh*128+p] + W4a[g, hh*128+p, 64]*r2_host[g]
    # aux2 = [b4bt(64) | noise2(64) | ones(1)]     -> [2, 129] f32
    w1t = nc.dram_tensor("w1t", [512, J * H], BF16, kind="ExternalInput").ap()
    w2t = nc.dram_tensor("w2t", [128, J * 2 * 64], BF16, kind="ExternalInput").ap()
    w4t = nc.dram_tensor("w4t", [128, (J // 2) * H], BF16, kind="ExternalInput").ap()
    w5t = nc.dram_tensor("w5t", [128, J * 4], BF16, kind="ExternalInput").ap()
    xbf = nc.dram_tensor("xbf", [128, 4], BF16, kind="ExternalInput").ap()
    aux1 = nc.dram_tensor("aux1", [128, 256], F32, kind="ExternalInput").ap()
    aux2 = nc.dram_tensor("aux2", [2, 129], F32, kind="ExternalInput").ap()
    out = nc.dram_tensor("out", [1, J], F32, kind="ExternalOutput").ap()

    NWCHUNK = 4  # stage-1 free-dim chunks of 4096 cols (1 MiB bf16 per DMA)

    with tile.TileContext(nc) as tc:
        with (
            tc.tile_pool(name="stream", bufs=8) as sp,
            tc.tile_pool(name="res", bufs=1) as rp,
            tc.tile_pool(name="psum", bufs=1, space=bass.MemorySpace.PSUM) as pp,
        ):
            # small resident tensors first (tiny; off the critical DMA path)
            xbf_t = rp.tile([128, 4], BF16)
            nc.sync.dma_start(xbf_t[:, :], xbf)
            aux1_t = rp.tile([128, 256], F32)
            nc.sync.dma_start(aux1_t[:, :], aux1)
            aux2_t = rp.tile([2, 129], F32)
            nc.sync.dma_start(aux2_t[:, :], aux2)
            w5s = rp.tile([128, J * 4], BF16)
            nc.sync.dma_start(w5s[:, :], w5t)

            corrt = aux1_t[:, 0:128]
            bias4t = aux1_t[:, 128:256]
            b4bt = aux2_t[:, 0:64]
            noise2 = aux2_t[:, 64:128]
            ones2 = aux2_t[:, 128:129]

            w2s = rp.tile([128, J * 2 * 64], BF16)
            w4s = rp.tile([128, (J // 2) * H], BF16)

            y1 = pp.tile([128, 128], F32)

            # ---- stage 1: y1[j,h] = sum_i W1a[g,h,i] x[i] ----
            # 128 psum columns t = 2j+hh ; 4 accumulating matmuls each (i-chunks)
            for u in range(NWCHUNK):
                tiles_u = []
                for ic in range(4):
                    wt = sp.tile([128, 4096], BF16, tag="w1s")
                    nc.sync.dma_start(
                        wt[:, :], w1t[128 * ic : 128 * (ic + 1), 4096 * u : 4096 * (u + 1)]
                    )
                    tiles_u.append(wt)
                for tt in range(32):
                    t = 32 * u + tt
                    for ic in range(4):
                        nc.tensor.matmul(
                            y1[:, t : t + 1],
                            tiles_u[ic][:, 128 * tt : 128 * (tt + 1)],
                            xbf_t[:, ic : ic + 1],
                            start=(ic == 0),
                            stop=(ic == 3),
                        )

            # stage-2/4 weights arrive after the w1t stream (sync ring is FIFO)
            for k in range(2):
                nc.sync.dma_start(
                    w2s[:, 4096 * k : 4096 * (k + 1)], w2t[:, 4096 * k : 4096 * (k + 1)]
                )
            for k in range(2):
                nc.sync.dma_start(
                    w4s[:, 4096 * k : 4096 * (k + 1)], w4t[:, 4096 * k : 4096 * (k + 1)]
                )

            # h1 = relu(y1 - corr)  (bf16 for the next stage's moving operand)
            h1f = rp.tile([128, 128], F32)
            nc.vector.tensor_sub(h1f[:, :], y1[:, :], corrt)
            h1sb = rp.tile([128, 128], BF16)
            nc.vector.tensor_scalar_max(h1sb[:, :], h1f[:, :], 0.0)

            # ---- stage 2: r1[j,m] = relu(sum_h W1b[g,m,h] h1[j,h]) ----
            # even j -> psum rows 0..63, odd j -> rows 64..127 (col-group tiling)
            psum2 = pp.tile([128, J], F32)
            nc.vector.memset(psum2[:, :], 0.0)
            for j in range(J):
                off = 64 * (j % 2)
                for hc in range(2):
                    nc.tensor.matmul(
                        psum2[off : off + 64, j : j + 1],
                        w2s[:, j * 128 + hc * 64 : j * 128 + hc * 64 + 64],
                        h1sb[:, 2 * j + hc : 2 * j + hc + 1],
                        start=(hc == 0),
                        stop=(hc == 1),
                    )
            r1cols = rp.tile([128, J], BF16)
            nc.vector.tensor_scalar_max(r1cols[:, :], psum2[:, :], 0.0)

            # ---- stage 4: h4[j,h] = relu(sum_m W4a[g,h,m] r1[j,m] + bias4eff) ----
            psum4 = pp.tile([128, 128], F32)
            for j in range(J):
                off = 64 * (j % 2)
                u_ = j // 2
                for hc in range(2):
                    nc.tensor.matmul(
                        psum4[:, 2 * j + hc : 2 * j + hc + 1],
                        w4s[off : off + 64, u_ * 256 + hc * 128 : u_ * 256 + hc * 128 + 128],
                        r1cols[off : off + 64, j : j + 1],
                        start=True,
                        stop=True,
                    )
            h4f = rp.tile([128, 128], F32)
            nc.vector.tensor_add(h4f[:, :], psum4[:, :], bias4t)
            h4cols = rp.tile([128, 128], BF16)
            nc.vector.tensor_scalar_max(h4cols[:, :], h4f[:, :], 0.0)

            # ---- stage 5: pred[o,j] = relu(sum_h W4b[g,o,h] h4[j,h] + b4b) ----
            psum5 = pp.tile([2, J], F32)
            for j in range(J):
                for hc in range(2):
                    nc.tensor.matmul(
                        psum5[0:2, j : j + 1],
                        w5s[:, j * 4 + hc * 2 : j * 4 + hc * 2 + 2],
                        h4cols[:, 2 * j + hc : 2 * j + hc + 1],
                        start=(hc == 0),
                        stop=(hc == 1),
                    )
            pred = rp.tile([2, J], F32)
            nc.vector.tensor_add(pred[:, :], psum5[:, :], b4bt)
            nc.vector.tensor_scalar_max(pred[:, :], pred[:, :], 0.0)

            # out = pred[0] + noise*pred[1]  (partition-sum via K=2 f32 matmul)
            scaled = rp.tile([2, J], F32)
            nc.vector.tensor_mul(scaled[:, :], pred[:, :], noise2)
            psum6 = pp.tile([1, J], F32)
            nc.tensor.matmul(psum6[0:1, :], ones2, scaled[:, :], start=True, stop=True)
            yrow = rp.tile([1, J], F32)
            nc.vector.tensor_copy(yrow[0:1, :], psum6[0:1, :])
            nc.sync.dma_start(out, yrow[0:1, :])

    nc.compile()
    _CACHE["nc"] = nc
    return nc


def _bf(a):
    return np.ascontiguousarray(a.astype(NP_BF16))


def prep_core_inputs(inputs, c):
    """Host-side shard + layout prep for core c. inputs are np float32 arrays."""
    x = inputs["x"][0]  # [512]
    gi = np.arange(J * c, J * (c + 1))
    xg = x[gi]
    jj = np.arange(J)

    W1a_c = inputs["W1a"][gi]  # [64, 256, 512]
    w1t = _bf(W1a_c.transpose(2, 0, 1).reshape(512, J * H))

    # self-excite path entirely on host (tiny), folded into stage-4 bias
    dW2 = inputs["W2a"][gi, :, gi]  # [64, 2]
    h2 = np.maximum(dW2 * xg[:, None] + inputs["b2a"][gi], 0.0)
    r2 = np.maximum((inputs["W2b"][gi, 0] * h2).sum(-1) + inputs["b2b"][gi, 0], 0.0)

    # correction computed from the bf16-rounded operands (exact cancellation
    # of the diagonal term the device's bf16 matmul actually added)
    dW1 = W1a_c[jj, :, gi].astype(NP_BF16).astype(np.float32)  # [64, 256]
    xg_bf = xg.astype(NP_BF16).astype(np.float32)
    corr_jh = dW1 * xg_bf[:, None]
    corrt = corr_jh.reshape(J, 2, 128).transpose(2, 0, 1).reshape(128, 128)

    W4a_c = inputs["W4a"][gi]  # [64, 256, 65]
    bias4_jh = inputs["b4a"][gi] + W4a_c[:, :, 64] * r2[:, None]
    bias4t = bias4_jh.reshape(J, 2, 128).transpose(2, 0, 1).reshape(128, 128)

    aux1 = np.ascontiguousarray(
        np.concatenate([corrt, bias4t], axis=1), dtype=np.float32
    )
    xbf = _bf(x.reshape(4, 128).T)  # [128, 4]

    b4bt = inputs["b4b"][gi].T  # [2, 64]
    noise2 = np.stack([np.ones(J, np.float32), inputs["noise"][gi]])
    ones2 = np.ones((2, 1), np.float32)
    aux2 = np.ascontiguousarray(
        np.concatenate([b4bt, noise2, ones2], axis=1), dtype=np.float32
    )

    W1b_c = inputs["W1b"][gi]  # [64, 64, 256]
    w2t = _bf(
        W1b_c.reshape(J, 64, 2, 128).transpose(3, 0, 2, 1).reshape(128, J * 2 * 64)
    )

    # stage 4: branch pairs stacked on partitions (even j -> rows 0..63)
    W4m = W4a_c[:, :, 0:64]  # [j, h, m]
    T4 = W4m.reshape(J // 2, 2, H, 64)  # [u, par, h, m]
    w4t = _bf(T4.transpose(1, 3, 0, 2).reshape(128, (J // 2) * H))

    W4b_c = inputs["W4b"][gi]  # [64, 2, 256]
    w5t = _bf(W4b_c.reshape(J, 2, 2, 128).transpose(3, 0, 2, 1).reshape(128, J * 4))

    return {
        "w1t": w1t, "w2t": w2t, "w4t": w4t, "w5t": w5t,
        "xbf": xbf, "aux1": aux1, "aux2": aux2,
    }


def run(inputs, trace=False, **kw):
    inputs = {k: np.asarray(v, dtype=np.float32) for k, v in inputs.items()}
    nc = _build_nc()
    in_maps = [prep_core_inputs(inputs, c) for c in range(NCORES)]
    res = run_bass_kernel_spmd(
        nc, in_maps, core_ids=list(range(NCORES)), trace=trace, **kw
    )
    out = np.concatenate([res.results[c]["out"] for c in range(NCORES)], axis=1)
    return out.astype(np.float32), res


def kernel(**inputs):
    out, _ = run(inputs)
    return out


# revision 5
# speedup vs baseline: 3.6240x; 1.0021x over previous
"""Trainium2 Bass kernel for CausalTrajectoryPrediction (batched per-branch MLPs).

Math (per branch n of N=512, H=256, M=64):
    x_off = x with element n zeroed ; x_diag = only element n kept
    h1 = relu(W1a[n] @ x_off)            # [H]
    r1 = relu(W1b[n] @ h1)               # [M]
    r2 = relu(W2b[n] @ relu(W2a[n] @ x_diag + b2a[n]) + b2b[n])   # [1]
    h4 = relu(W4a[n] @ [r1; r2] + b4a[n])                          # [H]
    pred = relu(W4b[n] @ h4 + b4b[n])                              # [2]
    out[n] = pred[0] + noise[n] * pred[1]

Device strategy (8 cores, 64 branches each, expert-parallel):
  * W1a[n] @ x_off == W1a[n] @ x - W1a[n,:,n]*x[n]; the shared matvec is done
    on-device (weights stationary, x moving), the diagonal correction is a
    tiny host-side gather shipped as a [128,128] f32 tile (computed from the
    bf16-rounded operands so the subtraction cancels exactly).
  * The self-excite path (r2) only touches W2a's diagonal -> computed on host
    (512 branches x ~8 flops) and folded into an effective stage-4 bias:
    bias4_eff = b4a + W4a[:,:,64] * r2.  Stage 4 then contracts over m=0..63.
  * Weights + moving activations in bf16 (PSUM accumulation f32, all bias /
    correction math f32): f32 matmuls lower to 2x LDWEIGHTS+MATMUL passes on
    TRN2, and bf16 stationaries get fast-weight-load; bf16 also halves the
    32MB weight stream.
  * All weights are pre-transposed on host into [K-partition, free] layouts so
    each per-branch matvec is a single matmul with the activation vector as
    the moving operand; outputs land on PSUM partitions in exactly the layout
    the next stage consumes.
  * DMAs all issue on the sync (HWDGE/SP) ring -> FIFO in program order:
    the W1a stream first (paces stage-1 PE), then stage-2/4 weights arrive
    just-in-time for the tail stages.
"""

import ml_dtypes
import numpy as np

import concourse.bacc as bacc
import concourse.bass as bass
import concourse.mybir as mybir
import concourse.tile as tile
from concourse.bass_utils import run_bass_kernel_spmd

F32 = mybir.dt.float32
BF16 = mybir.dt.float16
NP_BF16 = np.float16
NCORES = 8
N, H, M = 512, 256, 64
J = N // NCORES  # 64 branches per core

_CACHE = {}


def _build_nc():
    if "nc" in _CACHE:
        return _CACHE["nc"]

    nc = bacc.Bacc(
        "TRN2", target_bir_lowering=False, debug=False, enable_asserts=False,
        num_devices=NCORES,
    )

    # --- DRAM I/O (per-core shapes) ---
    # w1t[i, j*256+h]              = W1a[g, h, i]           (g = 64*core + j)
    # w2t[hl, j*128+hc*64+m]       = W1b[g, m, hc*128+hl]
    # w4t[64*(j%2)+m, (j//2)*256+h]= W4a[g, h, m]   (m<64; col 64 folded in bias)
    # w5t[hl, j*4+hc*2+o]          = W4b[g, o, hc*128+hl]
    # xbf[p, ic] = x[128*ic+p]                     (bf16 moving operand)
    # aux1 = [corr(128) | bias4(128)]   -> [128, 256] f32
    #   corr[p, 2j+hh]  = bf16(W1a[g, hh*128+p, g]) * bf16(x[g])
    #   bias4[p, 2j+hh] = b4a[g, hh*128+p] + W4a[g, hh*128+p, 64]*r2_host[g]
    # aux2 = [b4bt(64) | noise2(64) | ones(1)]     -> [2, 129] f32
    w1t = nc.dram_tensor("w1t", [512, J * H], BF16, kind="ExternalInput").ap()
    w2t = nc.dram_tensor("w2t", [128, J * 2 * 64], BF16, kind="ExternalInput").ap()
    w4t = nc.dram_tensor("w4t", [128, (J // 2) * H], BF16, kind="ExternalInput").ap()
    w5t = nc.dram_tensor("w5t", [128, J * 4], BF16, kind="ExternalInput").ap()
    xbf = nc.dram_tensor("xbf", [128, 4], BF16, kind="ExternalInput").ap()
    aux1 = nc.dram_tensor("aux1", [128, 256], F32, kind="ExternalInput").ap()
    aux2 = nc.dram_tensor("aux2", [2, 129], F32, kind="ExternalInput").ap()
    out = nc.dram_tensor("out", [1, J], F32, kind="ExternalOutput").ap()

    NWCHUNK = 8  # stage-1 free-dim chunks of 2048 cols (512 KiB fp16 per DMA)

    with tile.TileContext(nc) as tc:
        with (
            tc.tile_pool(name="stream", bufs=24) as sp,
            tc.tile_pool(name="res", bufs=1) as rp,
            tc.tile_pool(name="psum", bufs=1, space=bass.MemorySpace.PSUM) as pp,
        ):
            # small resident tensors on the scalar (ACT) HWDGE ring so they
            # don't delay the w1t stream on the sync ring
            xbf_t = rp.tile([128, 4], BF16)
            nc.scalar.dma_start(xbf_t[:, :], xbf)
            aux1_t = rp.tile([128, 256], F32)
            nc.scalar.dma_start(aux1_t[:, :], aux1)
            aux2_t = rp.tile([2, 129], F32)
            nc.scalar.dma_start(aux2_t[:, :], aux2)
            w5s = rp.tile([128, J * 4], BF16)
            nc.scalar.dma_start(w5s[:, :], w5t)

            corrt = aux1_t[:, 0:128]
            bias4t = aux1_t[:, 128:256]
            b4bt = aux2_t[:, 0:64]
            noise2 = aux2_t[:, 64:128]
            ones2 = aux2_t[:, 128:129]

            w2s = rp.tile([128, J * 2 * 64], BF16)
            w4s = rp.tile([128, (J // 2) * H], BF16)

            y1 = pp.tile([128, 128], F32)

            # ---- stage 1: y1[j,h] = sum_i W1a[g,h,i] x[i] ----
            # 128 psum columns t = 2j+hh ; 4 accumulating matmuls each (i-chunks)
            for u in range(NWCHUNK):
                tiles_u = []
                for ic in range(4):
                    wt = sp.tile([128, 2048], BF16, tag="w1s")
                    nc.sync.dma_start(
                        wt[:, :], w1t[128 * ic : 128 * (ic + 1), 2048 * u : 2048 * (u + 1)]
                    )
                    tiles_u.append(wt)
                for tt in range(16):
                    t = 16 * u + tt
                    for ic in range(4):
                        nc.tensor.matmul(
                            y1[:, t : t + 1],
                            tiles_u[ic][:, 128 * tt : 128 * (tt + 1)],
                            xbf_t[:, ic : ic + 1],
                            start=(ic == 0),
                            stop=(ic == 3),
                        )

            # stage-2/4 weights arrive after the w1t stream (sync ring is FIFO)
            for k in range(2):
                nc.sync.dma_start(
                    w2s[:, 4096 * k : 4096 * (k + 1)], w2t[:, 4096 * k : 4096 * (k + 1)]
                )
            for k in range(2):
                nc.sync.dma_start(
                    w4s[:, 4096 * k : 4096 * (k + 1)], w4t[:, 4096 * k : 4096 * (k + 1)]
                )

            # h1 = relu(y1 - corr)  (bf16 for the next stage's moving operand)
            h1f = rp.tile([128, 128], F32)
            nc.vector.tensor_sub(h1f[:, :], y1[:, :], corrt)
            h1sb = rp.tile([128, 128], BF16)
            nc.vector.tensor_scalar_max(h1sb[:, :], h1f[:, :], 0.0)

            # ---- stage 2: r1[j,m] = relu(sum_h W1b[g,m,h] h1[j,h]) ----
            # even j -> psum rows 0..63, odd j -> rows 64..127 (col-group tiling)
            psum2 = pp.tile([128, J], F32)
            nc.vector.memset(psum2[:, :], 0.0)
            for j in range(J):
                off = 64 * (j % 2)
                for hc in range(2):
                    nc.tensor.matmul(
                        psum2[off : off + 64, j : j + 1],
                        w2s[:, j * 128 + hc * 64 : j * 128 + hc * 64 + 64],
                        h1sb[:, 2 * j + hc : 2 * j + hc + 1],
                        start=(hc == 0),
                        stop=(hc == 1),
                    )
            r1cols = rp.tile([128, J], BF16)
            nc.vector.tensor_scalar_max(r1cols[:, :], psum2[:, :], 0.0)

            # ---- stage 4: h4[j,h] = relu(sum_m W4a[g,h,m] r1[j,m] + bias4eff) ----
            psum4 = pp.tile([128, 128], F32)
            for j in range(J):
                off = 64 * (j % 2)
                u_ = j // 2
                for hc in range(2):
                    nc.tensor.matmul(
                        psum4[:, 2 * j + hc : 2 * j + hc + 1],
                        w4s[off : off + 64, u_ * 256 + hc * 128 : u_ * 256 + hc * 128 + 128],
                        r1cols[off : off + 64, j : j + 1],
                        start=True,
                        stop=True,
                    )
            h4f = rp.tile([128, 128], F32)
            nc.vector.tensor_add(h4f[:, :], psum4[:, :], bias4t)
            h4cols = rp.tile([128, 128], BF16)
            nc.vector.tensor_scalar_max(h4cols[:, :], h4f[:, :], 0.0)

            # ---- stage 5: pred[o,j] = relu(sum_h W4b[g,o,h] h4[j,h] + b4b) ----
            psum5 = pp.tile([2, J], F32)
            for j in range(J):
                for hc in range(2):
                    nc.tensor.matmul(
                        psum5[0:2, j : j + 1],
                        w5s[:, j * 4 + hc * 2 : j * 4 + hc * 2 + 2],
                        h4cols[:, 2 * j + hc : 2 * j + hc + 1],
                        start=(hc == 0),
                        stop=(hc == 1),
                    )
            pred = rp.tile([2, J], F32)
            nc.vector.tensor_add(pred[:, :], psum5[:, :], b4bt)
            nc.vector.tensor_scalar_max(pred[:, :], pred[:, :], 0.0)

            # out = pred[0] + noise*pred[1]  (partition-sum via K=2 f32 matmul)
            scaled = rp.tile([2, J], F32)
            nc.vector.tensor_mul(scaled[:, :], pred[:, :], noise2)
            psum6 = pp.tile([1, J], F32)
            nc.tensor.matmul(psum6[0:1, :], ones2, scaled[:, :], start=True, stop=True)
            yrow = rp.tile([1, J], F32)
            nc.vector.tensor_copy(yrow[0:1, :], psum6[0:1, :])
            nc.sync.dma_start(out, yrow[0:1, :])

    nc.compile()
    _CACHE["nc"] = nc
    return nc


def _bf(a):
    return np.ascontiguousarray(a.astype(NP_BF16))


def prep_core_inputs(inputs, c):
    """Host-side shard + layout prep for core c. inputs are np float32 arrays."""
    x = inputs["x"][0]  # [512]
    gi = np.arange(J * c, J * (c + 1))
    xg = x[gi]
    jj = np.arange(J)

    W1a_c = inputs["W1a"][gi]  # [64, 256, 512]
    w1t = _bf(W1a_c.transpose(2, 0, 1).reshape(512, J * H))

    # self-excite path entirely on host (tiny), folded into stage-4 bias
    dW2 = inputs["W2a"][gi, :, gi]  # [64, 2]
    h2 = np.maximum(dW2 * xg[:, None] + inputs["b2a"][gi], 0.0)
    r2 = np.maximum((inputs["W2b"][gi, 0] * h2).sum(-1) + inputs["b2b"][gi, 0], 0.0)

    # correction computed from the bf16-rounded operands (exact cancellation
    # of the diagonal term the device's bf16 matmul actually added)
    dW1 = W1a_c[jj, :, gi].astype(NP_BF16).astype(np.float32)  # [64, 256]
    xg_bf = xg.astype(NP_BF16).astype(np.float32)
    corr_jh = dW1 * xg_bf[:, None]
    corrt = corr_jh.reshape(J, 2, 128).transpose(2, 0, 1).reshape(128, 128)

    W4a_c = inputs["W4a"][gi]  # [64, 256, 65]
    bias4_jh = inputs["b4a"][gi] + W4a_c[:, :, 64] * r2[:, None]
    bias4t = bias4_jh.reshape(J, 2, 128).transpose(2, 0, 1).reshape(128, 128)

    aux1 = np.ascontiguousarray(
        np.concatenate([corrt, bias4t], axis=1), dtype=np.float32
    )
    xbf = _bf(x.reshape(4, 128).T)  # [128, 4]

    b4bt = inputs["b4b"][gi].T  # [2, 64]
    noise2 = np.stack([np.ones(J, np.float32), inputs["noise"][gi]])
    ones2 = np.ones((2, 1), np.float32)
    aux2 = np.ascontiguousarray(
        np.concatenate([b4bt, noise2, ones2], axis=1), dtype=np.float32
    )

    W1b_c = inputs["W1b"][gi]  # [64, 64, 256]
    w2t = _bf(
        W1b_c.reshape(J, 64, 2, 128).transpose(3, 0, 2, 1).reshape(128, J * 2 * 64)
    )

    # stage 4: branch pairs stacked on partitions (even j -> rows 0..63)
    W4m = W4a_c[:, :, 0:64]  # [j, h, m]
    T4 = W4m.reshape(J // 2, 2, H, 64)  # [u, par, h, m]
    w4t = _bf(T4.transpose(1, 3, 0, 2).reshape(128, (J // 2) * H))

    W4b_c = inputs["W4b"][gi]  # [64, 2, 256]
    w5t = _bf(W4b_c.reshape(J, 2, 2, 128).transpose(3, 0, 2, 1).reshape(128, J * 4))

    return {
        "w1t": w1t, "w2t": w2t, "w4t": w4t, "w5t": w5t,
        "xbf": xbf, "aux1": aux1, "aux2": aux2,
    }


def run(inputs, trace=False, **kw):
    inputs = {k: np.asarray(v, dtype=np.float32) for k, v in inputs.items()}
    nc = _build_nc()
    in_maps = [prep_core_inputs(inputs, c) for c in range(NCORES)]
    res = run_bass_kernel_spmd(
        nc, in_maps, core_ids=list(range(NCORES)), trace=trace, **kw
    )
    out = np.concatenate([res.results[c]["out"] for c in range(NCORES)], axis=1)
    return out.astype(np.float32), res


def kernel(**inputs):
    out, _ = run(inputs)
    return out


# revision 12
# speedup vs baseline: 3.7237x; 1.0275x over previous
"""Trainium2 Bass kernel for CausalTrajectoryPrediction (batched per-branch MLPs).

Math (per branch n of N=512, H=256, M=64):
    x_off = x with element n zeroed ; x_diag = only element n kept
    h1 = relu(W1a[n] @ x_off)            # [H]
    r1 = relu(W1b[n] @ h1)               # [M]
    r2 = relu(W2b[n] @ relu(W2a[n] @ x_diag + b2a[n]) + b2b[n])   # [1]
    h4 = relu(W4a[n] @ [r1; r2] + b4a[n])                          # [H]
    pred = relu(W4b[n] @ h4 + b4b[n])                              # [2]
    out[n] = pred[0] + noise[n] * pred[1]

Device strategy (8 cores, 64 branches each, expert-parallel):
  * W1a[n] @ x_off == W1a[n] @ x - W1a[n,:,n]*x[n]; the shared matvec is done
    on-device (weights stationary, x moving), the diagonal correction is a
    tiny host-side gather shipped as a [128,128] f32 tile (computed from the
    bf16-rounded operands so the subtraction cancels exactly).
  * The self-excite path (r2) only touches W2a's diagonal -> computed on host
    (512 branches x ~8 flops) and folded into an effective stage-4 bias:
    bias4_eff = b4a + W4a[:,:,64] * r2.  Stage 4 then contracts over m=0..63.
  * Weights + moving activations in bf16 (PSUM accumulation f32, all bias /
    correction math f32): f32 matmuls lower to 2x LDWEIGHTS+MATMUL passes on
    TRN2, and bf16 stationaries get fast-weight-load; bf16 also halves the
    32MB weight stream.
  * All weights are pre-transposed on host into [K-partition, free] layouts so
    each per-branch matvec is a single matmul with the activation vector as
    the moving operand; outputs land on PSUM partitions in exactly the layout
    the next stage consumes.
  * DMAs all issue on the sync (HWDGE/SP) ring -> FIFO in program order:
    the W1a stream first (paces stage-1 PE), then stage-2/4 weights arrive
    just-in-time for the tail stages.
"""

import ml_dtypes
import numpy as np

import concourse.bacc as bacc
import concourse.bass as bass
import concourse.mybir as mybir
import concourse.tile as tile
from concourse.bass_utils import run_bass_kernel_spmd

F32 = mybir.dt.float32
BF16 = mybir.dt.float16
NP_BF16 = np.float16
NCORES = 8
N, H, M = 512, 256, 64
J = N // NCORES  # 64 branches per core

_CACHE = {}


def _build_nc():
    if "nc" in _CACHE:
        return _CACHE["nc"]

    nc = bacc.Bacc(
        "TRN2", target_bir_lowering=False, debug=False, enable_asserts=False,
        num_devices=NCORES,
    )

    # --- DRAM I/O (per-core shapes) ---
    # w1t[i, j*256+h]              = W1a[g, h, i]           (g = 64*core + j)
    # w2t[hl, j*128+hc*64+m]       = W1b[g, m, hc*128+hl]
    # w4t[64*(j%2)+m, (j//2)*256+h]= W4a[g, h, m]   (m<64; col 64 folded in bias)
    # w5t[hl, j*4+hc*2+o]          = W4b[g, o, hc*128+hl]
    # xbf[p, ic] = x[128*ic+p]                     (bf16 moving operand)
    # aux1 = [corr(128) | bias4(128)]   -> [128, 256] f32
    #   corr[p, 2j+hh]  = bf16(W1a[g, hh*128+p, g]) * bf16(x[g])
    #   bias4[p, 2j+hh] = b4a[g, hh*128+p] + W4a[g, hh*128+p, 64]*r2_host[g]
    # aux2 = [b4bt(64) | noise2(64) | ones(1)]     -> [2, 129] f32
    w1t = nc.dram_tensor("w1t", [512, J * H], BF16, kind="ExternalInput").ap()
    w2t = nc.dram_tensor("w2t", [128, J * 2 * 64], BF16, kind="ExternalInput").ap()
    w4t = nc.dram_tensor("w4t", [128, (J // 2) * H], BF16, kind="ExternalInput").ap()
    w5t = nc.dram_tensor("w5t", [128, J * 4], BF16, kind="ExternalInput").ap()
    xbf = nc.dram_tensor("xbf", [128, 4], BF16, kind="ExternalInput").ap()
    aux1 = nc.dram_tensor("aux1", [128, 256], F32, kind="ExternalInput").ap()
    # aux3 = [dmask(32) | msk2(64) | b4bp(2)] -> [64, 98] f32
    #   dmask[2jl+o, jf] = (jl==jf)
    #   msk2[2jl+o, 32g+jf] = (jl==jf) * (1 if o==0 else noise[64c+32g+jf])
    #   b4bp[2jl+o, g] = b4b[64c+32g+jl, o]
    aux3 = nc.dram_tensor("aux3", [64, 98], F32, kind="ExternalInput").ap()
    out = nc.dram_tensor("out", [1, J], F32, kind="ExternalOutput").ap()

    NWCHUNK = 8  # stage-1 free-dim chunks of 2048 cols (512 KiB fp16 per DMA)

    with tile.TileContext(nc) as tc:
        with (
            tc.tile_pool(name="stream", bufs=24) as sp,
            tc.tile_pool(name="res", bufs=1) as rp,
            tc.tile_pool(name="psum", bufs=1, space=bass.MemorySpace.PSUM) as pp,
        ):
            # small resident tensors on the scalar (ACT) HWDGE ring so they
            # don't delay the w1t stream on the sync ring
            xbf_t = rp.tile([128, 4], BF16)
            nc.scalar.dma_start(xbf_t[:, :], xbf)
            aux1_t = rp.tile([128, 256], F32)
            nc.scalar.dma_start(aux1_t[:, :], aux1)
            aux3_t = rp.tile([64, 98], F32)
            nc.scalar.dma_start(aux3_t[:, :], aux3)
            w5s = rp.tile([128, J * 4], BF16)
            nc.scalar.dma_start(w5s[:, :], w5t)

            corrt = aux1_t[:, 0:128]
            bias4t = aux1_t[:, 128:256]
            dmask = aux3_t[:, 0:32]
            msk2 = aux3_t[:, 32:96]
            b4bp = aux3_t[:, 96:98]

            w2s = rp.tile([128, J * 2 * 64], BF16)
            w4s = rp.tile([128, (J // 2) * H], BF16)

            y1 = pp.tile([128, 128], F32)

            # ---- stage 1: y1[j,h] = sum_i W1a[g,h,i] x[i] ----
            # 128 psum columns t = 2j+hh ; 4 accumulating matmuls each (i-chunks)
            for u in range(NWCHUNK):
                tiles_u = []
                for ic in range(4):
                    wt = sp.tile([128, 2048], BF16, tag="w1s")
                    nc.sync.dma_start(
                        wt[:, :], w1t[128 * ic : 128 * (ic + 1), 2048 * u : 2048 * (u + 1)]
                    )
                    tiles_u.append(wt)
                for tt in range(16):
                    t = 16 * u + tt
                    for ic in range(4):
                        nc.tensor.matmul(
                            y1[:, t : t + 1],
                            tiles_u[ic][:, 128 * tt : 128 * (tt + 1)],
                            xbf_t[:, ic : ic + 1],
                            start=(ic == 0),
                            stop=(ic == 3),
                        )

            # stage-2/4 weights arrive after the w1t stream (sync ring is FIFO);
            # fine 512KB chunks so the tail consumes them just-in-time
            for k in range(4):
                nc.sync.dma_start(
                    w2s[:, 2048 * k : 2048 * (k + 1)], w2t[:, 2048 * k : 2048 * (k + 1)]
                )
            for k in range(4):
                nc.sync.dma_start(
                    w4s[:, 2048 * k : 2048 * (k + 1)], w4t[:, 2048 * k : 2048 * (k + 1)]
                )

            # h1 = relu(y1 - corr)  (bf16 for the next stage's moving operand)
            h1f = rp.tile([128, 128], F32)
            nc.vector.tensor_sub(h1f[:, :], y1[:, :], corrt)
            h1sb = rp.tile([128, 128], BF16)
            nc.vector.tensor_scalar_max(h1sb[:, :], h1f[:, :], 0.0)

            # ---- stage 2: r1[j,m] = relu(sum_h W1b[g,m,h] h1[j,h]) ----
            # even j -> psum rows 0..63, odd j -> rows 64..127 (col-group tiling)
            psum2 = pp.tile([128, J], F32)
            nc.vector.memset(psum2[:, :], 0.0)
            for j in range(J):
                off = 64 * (j % 2)
                for hc in range(2):
                    nc.tensor.matmul(
                        psum2[off : off + 64, j : j + 1],
                        w2s[:, j * 128 + hc * 64 : j * 128 + hc * 64 + 64],
                        h1sb[:, 2 * j + hc : 2 * j + hc + 1],
                        start=(hc == 0),
                        stop=(hc == 1),
                    )
            r1cols = rp.tile([128, J], BF16)
            nc.vector.tensor_scalar_max(r1cols[:, :], psum2[:, :], 0.0)

            # ---- stage 4: h4[j,h] = relu(sum_m W4a[g,h,m] r1[j,m] + bias4eff) ----
            # one K=128 stationary per branch PAIR (the wrong-parity half of
            # each r1cols column is exactly zero), h4 col layout = hc*64 + j
            psum4 = pp.tile([128, 128], F32)
            for u_ in range(J // 2):
                for hc in range(2):
                    for par in range(2):
                        j = 2 * u_ + par
                        nc.tensor.matmul(
                            psum4[:, hc * 64 + j : hc * 64 + j + 1],
                            w4s[:, u_ * 256 + hc * 128 : u_ * 256 + hc * 128 + 128],
                            r1cols[:, j : j + 1],
                            start=True,
                            stop=True,
                        )
            h4f = rp.tile([128, 128], F32)
            nc.vector.tensor_add(h4f[:, :], psum4[:, :], bias4t)
            h4cols = rp.tile([128, 128], BF16)
            nc.vector.tensor_scalar_max(h4cols[:, :], h4f[:, :], 0.0)

            # ---- stage 5: 32 branches per stationary, F=32 moving ----
            # psum5g[(2jl+o), 32g+jf] = sum_h W4b[32g+jl, o, h] h4[32g+jf, h]
            psum5g = pp.tile([64, 64], F32)
            for grp in range(2):
                for hc in range(2):
                    nc.tensor.matmul(
                        psum5g[0:64, 32 * grp : 32 * grp + 32],
                        w5s[:, 128 * grp + 64 * hc : 128 * grp + 64 * hc + 64],
                        h4cols[:, 64 * hc + 32 * grp : 64 * hc + 32 * grp + 32],
                        start=(hc == 0),
                        stop=(hc == 1),
                    )
            # diag-extract via mask+reduce, then bias+relu -> predcol[(jl,o), g]
            ttr_dump = rp.tile([64, 64], F32)
            predcol = rp.tile([64, 2], F32)
            for grp in range(2):
                nc.vector.tensor_mul(
                    ttr_dump[:, 32 * grp : 32 * grp + 32],
                    psum5g[0:64, 32 * grp : 32 * grp + 32],
                    dmask,
                )
                nc.vector.tensor_reduce(
                    predcol[:, grp : grp + 1],
                    ttr_dump[:, 32 * grp : 32 * grp + 32],
                    mybir.AxisListType.X,
                    mybir.AluOpType.add,
                )
            nc.vector.tensor_add(predcol[:, :], predcol[:, :], b4bp)
            nc.vector.tensor_scalar_max(predcol[:, :], predcol[:, :], 0.0)

            # out[32g+jf] = sum_{jl,o} predcol[(jl,o), g] * msk2[(jl,o), 32g+jf]
            psum6 = pp.tile([1, J], F32)
            for grp in range(2):
                nc.tensor.matmul(
                    psum6[0:1, 32 * grp : 32 * grp + 32],
                    predcol[:, grp : grp + 1],
                    msk2[:, 32 * grp : 32 * grp + 32],
                    start=True,
                    stop=True,
                )
            yrow = rp.tile([1, J], F32)
            nc.vector.tensor_copy(yrow[0:1, :], psum6[0:1, :])
            nc.sync.dma_start(out, yrow[0:1, :])

    nc.compile()
    _CACHE["nc"] = nc
    return nc


def _bf(a):
    return np.ascontiguousarray(a.astype(NP_BF16))


def prep_core_inputs(inputs, c):
    """Host-side shard + layout prep for core c. inputs are np float32 arrays."""
    x = inputs["x"][0]  # [512]
    gi = np.arange(J * c, J * (c + 1))
    xg = x[gi]
    jj = np.arange(J)

    W1a_c = inputs["W1a"][gi]  # [64, 256, 512]
    w1t = _bf(W1a_c.transpose(2, 0, 1).reshape(512, J * H))

    # self-excite path entirely on host (tiny), folded into stage-4 bias
    dW2 = inputs["W2a"][gi, :, gi]  # [64, 2]
    h2 = np.maximum(dW2 * xg[:, None] + inputs["b2a"][gi], 0.0)
    r2 = np.maximum((inputs["W2b"][gi, 0] * h2).sum(-1) + inputs["b2b"][gi, 0], 0.0)

    # correction computed from the bf16-rounded operands (exact cancellation
    # of the diagonal term the device's bf16 matmul actually added)
    dW1 = W1a_c[jj, :, gi].astype(NP_BF16).astype(np.float32)  # [64, 256]
    xg_bf = xg.astype(NP_BF16).astype(np.float32)
    corr_jh = dW1 * xg_bf[:, None]
    corrt = corr_jh.reshape(J, 2, 128).transpose(2, 0, 1).reshape(128, 128)

    W4a_c = inputs["W4a"][gi]  # [64, 256, 65]
    bias4_jh = inputs["b4a"][gi] + W4a_c[:, :, 64] * r2[:, None]
    # h4 col layout = hc*64 + j
    bias4t = bias4_jh.reshape(J, 2, 128).transpose(2, 1, 0).reshape(128, 128)

    aux1 = np.ascontiguousarray(
        np.concatenate([corrt, bias4t], axis=1), dtype=np.float32
    )
    xbf = _bf(x.reshape(4, 128).T)  # [128, 4]

    # aux3: dmask | msk2 | b4bp   (stage-5 diag-extract + final combine)
    noise_c = inputs["noise"][gi]
    dmask = np.repeat(np.eye(32, dtype=np.float32), 2, axis=0)  # [64, 32]
    msk2 = np.zeros((64, 64), np.float32)
    for grp in range(2):
        jf = np.arange(32)
        msk2[2 * jf, 32 * grp + jf] = 1.0
        msk2[2 * jf + 1, 32 * grp + jf] = noise_c[32 * grp + jf]
    b4bp = (
        inputs["b4b"][gi].reshape(2, 32, 2).transpose(1, 2, 0).reshape(64, 2)
    )  # [2jl+o, grp]
    aux3 = np.ascontiguousarray(
        np.concatenate([dmask, msk2, b4bp], axis=1), dtype=np.float32
    )

    W1b_c = inputs["W1b"][gi]  # [64, 64, 256]
    w2t = _bf(
        W1b_c.reshape(J, 64, 2, 128).transpose(3, 0, 2, 1).reshape(128, J * 2 * 64)
    )

    # stage 4: branch pairs stacked on partitions (even j -> rows 0..63)
    W4m = W4a_c[:, :, 0:64]  # [j, h, m]
    T4 = W4m.reshape(J // 2, 2, H, 64)  # [u, par, h, m]
    w4t = _bf(T4.transpose(1, 3, 0, 2).reshape(128, (J // 2) * H))

    # w5p[hl, grp*128 + hc*64 + 2jl+o] = W4b[g(32grp+jl), o, hc*128+hl]
    W4b_c = inputs["W4b"][gi]  # [64, 2, 256]
    w5t = _bf(
        W4b_c.reshape(2, 32, 2, 2, 128).transpose(4, 0, 3, 1, 2).reshape(128, J * 4)
    )

    return {
        "w1t": w1t, "w2t": w2t, "w4t": w4t, "w5t": w5t,
        "xbf": xbf, "aux1": aux1, "aux3": aux3,
    }


def run(inputs, trace=False, **kw):
    inputs = {k: np.asarray(v, dtype=np.float32) for k, v in inputs.items()}
    nc = _build_nc()
    in_maps = [prep_core_inputs(inputs, c) for c in range(NCORES)]
    res = run_bass_kernel_spmd(
        nc, in_maps, core_ids=list(range(NCORES)), trace=trace, **kw
    )
    out = np.concatenate([res.results[c]["out"] for c in range(NCORES)], axis=1)
    return out.astype(np.float32), res


def kernel(**inputs):
    out, _ = run(inputs)
    return out


# revision 14
# speedup vs baseline: 3.7273x; 1.0010x over previous
"""Trainium2 Bass kernel for CausalTrajectoryPrediction (batched per-branch MLPs).

Math (per branch n of N=512, H=256, M=64):
    x_off = x with element n zeroed ; x_diag = only element n kept
    h1 = relu(W1a[n] @ x_off)            # [H]
    r1 = relu(W1b[n] @ h1)               # [M]
    r2 = relu(W2b[n] @ relu(W2a[n] @ x_diag + b2a[n]) + b2b[n])   # [1]
    h4 = relu(W4a[n] @ [r1; r2] + b4a[n])                          # [H]
    pred = relu(W4b[n] @ h4 + b4b[n])                              # [2]
    out[n] = pred[0] + noise[n] * pred[1]

Device strategy (8 cores, 64 branches each, expert-parallel):
  * W1a[n] @ x_off == W1a[n] @ x - W1a[n,:,n]*x[n]; the shared matvec is done
    on-device (weights stationary, x moving), the diagonal correction is a
    tiny host-side gather shipped as a [128,128] f32 tile (computed from the
    bf16-rounded operands so the subtraction cancels exactly).
  * The self-excite path (r2) only touches W2a's diagonal -> computed on host
    (512 branches x ~8 flops) and folded into an effective stage-4 bias:
    bias4_eff = b4a + W4a[:,:,64] * r2.  Stage 4 then contracts over m=0..63.
  * Weights + moving activations in bf16 (PSUM accumulation f32, all bias /
    correction math f32): f32 matmuls lower to 2x LDWEIGHTS+MATMUL passes on
    TRN2, and bf16 stationaries get fast-weight-load; bf16 also halves the
    32MB weight stream.
  * All weights are pre-transposed on host into [K-partition, free] layouts so
    each per-branch matvec is a single matmul with the activation vector as
    the moving operand; outputs land on PSUM partitions in exactly the layout
    the next stage consumes.
  * DMAs all issue on the sync (HWDGE/SP) ring -> FIFO in program order:
    the W1a stream first (paces stage-1 PE), then stage-2/4 weights arrive
    just-in-time for the tail stages.
"""

import ml_dtypes
import numpy as np

import concourse.bacc as bacc
import concourse.bass as bass
import concourse.mybir as mybir
import concourse.tile as tile
from concourse.bass_utils import run_bass_kernel_spmd

F32 = mybir.dt.float32
BF16 = mybir.dt.float16
NP_BF16 = np.float16
NCORES = 8
N, H, M = 512, 256, 64
J = N // NCORES  # 64 branches per core

_CACHE = {}


def _build_nc():
    if "nc" in _CACHE:
        return _CACHE["nc"]

    nc = bacc.Bacc(
        "TRN2", target_bir_lowering=False, debug=False, enable_asserts=False,
        num_devices=NCORES,
    )

    # --- DRAM I/O (per-core shapes) ---
    # w1t[i, j*256+h]              = W1a[g, h, i]           (g = 64*core + j)
    # w2t[hl, j*128+hc*64+m]       = W1b[g, m, hc*128+hl]
    # w4t[64*(j%2)+m, (j//2)*256+h]= W4a[g, h, m]   (m<64; col 64 folded in bias)
    # w5t[hl, j*4+hc*2+o]          = W4b[g, o, hc*128+hl]
    # xbf[p, ic] = x[128*ic+p]                     (bf16 moving operand)
    # aux1 = [corr(128) | bias4(128)]   -> [128, 256] f32
    #   corr[p, 2j+hh]  = bf16(W1a[g, hh*128+p, g]) * bf16(x[g])
    #   bias4[p, 2j+hh] = b4a[g, hh*128+p] + W4a[g, hh*128+p, 64]*r2_host[g]
    # aux2 = [b4bt(64) | noise2(64) | ones(1)]     -> [2, 129] f32
    w1t = nc.dram_tensor("w1t", [512, J * H], BF16, kind="ExternalInput").ap()
    w2t = nc.dram_tensor("w2t", [128, J * 2 * 64], BF16, kind="ExternalInput").ap()
    w4t = nc.dram_tensor("w4t", [128, (J // 2) * H], BF16, kind="ExternalInput").ap()
    w5t = nc.dram_tensor("w5t", [128, J * 4], BF16, kind="ExternalInput").ap()
    xbf = nc.dram_tensor("xbf", [128, 4], BF16, kind="ExternalInput").ap()
    aux1 = nc.dram_tensor("aux1", [128, 256], F32, kind="ExternalInput").ap()
    # aux3 = [dmask(32) | msk2(64) | b4bp(2)] -> [64, 98] f32
    #   dmask[2jl+o, jf] = (jl==jf)
    #   msk2[2jl+o, 32g+jf] = (jl==jf) * (1 if o==0 else noise[64c+32g+jf])
    #   b4bp[2jl+o, g] = b4b[64c+32g+jl, o]
    aux3 = nc.dram_tensor("aux3", [64, 98], F32, kind="ExternalInput").ap()
    out = nc.dram_tensor("out", [1, J], F32, kind="ExternalOutput").ap()

    # stage-1 free-dim chunk plan: small first chunk (fast rampup), 1 MiB rest
    CHUNK_COLS = [1024, 3072, 4096, 4096, 4096]

    with tile.TileContext(nc) as tc:
        with (
            tc.tile_pool(name="stream", bufs=12) as sp,
            tc.tile_pool(name="res", bufs=1) as rp,
            tc.tile_pool(name="psum", bufs=1, space=bass.MemorySpace.PSUM) as pp,
        ):
            # small resident tensors on the scalar (ACT) HWDGE ring so they
            # don't delay the w1t stream on the sync ring
            xbf_t = rp.tile([128, 4], BF16)
            nc.scalar.dma_start(xbf_t[:, :], xbf)
            aux1_t = rp.tile([128, 256], F32)
            nc.scalar.dma_start(aux1_t[:, :], aux1)
            aux3_t = rp.tile([64, 98], F32)
            nc.scalar.dma_start(aux3_t[:, :], aux3)
            w5s = rp.tile([128, J * 4], BF16)
            nc.scalar.dma_start(w5s[:, :], w5t)

            corrt = aux1_t[:, 0:128]
            bias4t = aux1_t[:, 128:256]
            dmask = aux3_t[:, 0:32]
            msk2 = aux3_t[:, 32:96]
            b4bp = aux3_t[:, 96:98]

            w2s = rp.tile([128, J * 2 * 64], BF16)
            w4s = rp.tile([128, (J // 2) * H], BF16)

            y1 = pp.tile([128, 128], F32)

            # ---- stage 1: y1[j,h] = sum_i W1a[g,h,i] x[i] ----
            # 128 psum columns t = 2j+hh ; 4 accumulating matmuls each (i-chunks)
            col0 = 0
            for ncols in CHUNK_COLS:
                tiles_u = []
                for ic in range(4):
                    wt = sp.tile([128, ncols], BF16, tag="w1s")
                    nc.sync.dma_start(
                        wt[:, :], w1t[128 * ic : 128 * (ic + 1), col0 : col0 + ncols]
                    )
                    tiles_u.append(wt)
                for tt in range(ncols // 128):
                    t = col0 // 128 + tt
                    for ic in range(4):
                        nc.tensor.matmul(
                            y1[:, t : t + 1],
                            tiles_u[ic][:, 128 * tt : 128 * (tt + 1)],
                            xbf_t[:, ic : ic + 1],
                            start=(ic == 0),
                            stop=(ic == 3),
                        )
                col0 += ncols

            # stage-2/4 weights arrive after the w1t stream (sync ring is FIFO);
            # fine 512KB chunks so the tail consumes them just-in-time
            for k in range(4):
                nc.sync.dma_start(
                    w2s[:, 2048 * k : 2048 * (k + 1)], w2t[:, 2048 * k : 2048 * (k + 1)]
                )
            for k in range(4):
                nc.sync.dma_start(
                    w4s[:, 2048 * k : 2048 * (k + 1)], w4t[:, 2048 * k : 2048 * (k + 1)]
                )

            # h1 = relu(y1 - corr)  (bf16 for the next stage's moving operand)
            h1f = rp.tile([128, 128], F32)
            nc.vector.tensor_sub(h1f[:, :], y1[:, :], corrt)
            h1sb = rp.tile([128, 128], BF16)
            nc.vector.tensor_scalar_max(h1sb[:, :], h1f[:, :], 0.0)

            # ---- stage 2: r1[j,m] = relu(sum_h W1b[g,m,h] h1[j,h]) ----
            # even j -> psum rows 0..63, odd j -> rows 64..127 (col-group tiling)
            psum2 = pp.tile([128, J], F32)
            nc.vector.memset(psum2[:, :], 0.0)
            for j in range(J):
                off = 64 * (j % 2)
                for hc in range(2):
                    nc.tensor.matmul(
                        psum2[off : off + 64, j : j + 1],
                        w2s[:, j * 128 + hc * 64 : j * 128 + hc * 64 + 64],
                        h1sb[:, 2 * j + hc : 2 * j + hc + 1],
                        start=(hc == 0),
                        stop=(hc == 1),
                    )
            r1cols = rp.tile([128, J], BF16)
            nc.vector.tensor_scalar_max(r1cols[:, :], psum2[:, :], 0.0)

            # ---- stage 4: h4[j,h] = relu(sum_m W4a[g,h,m] r1[j,m] + bias4eff) ----
            # one K=128 stationary per branch PAIR (the wrong-parity half of
            # each r1cols column is exactly zero), h4 col layout = hc*64 + j
            psum4 = pp.tile([128, 128], F32)
            for u_ in range(J // 2):
                for hc in range(2):
                    for par in range(2):
                        j = 2 * u_ + par
                        nc.tensor.matmul(
                            psum4[:, hc * 64 + j : hc * 64 + j + 1],
                            w4s[:, u_ * 256 + hc * 128 : u_ * 256 + hc * 128 + 128],
                            r1cols[:, j : j + 1],
                            start=True,
                            stop=True,
                        )
            h4f = rp.tile([128, 128], F32)
            nc.vector.tensor_add(h4f[:, :], psum4[:, :], bias4t)
            h4cols = rp.tile([128, 128], BF16)
            nc.vector.tensor_scalar_max(h4cols[:, :], h4f[:, :], 0.0)

            # ---- stage 5: 32 branches per stationary, F=32 moving ----
            # psum5g[(2jl+o), 32g+jf] = sum_h W4b[32g+jl, o, h] h4[32g+jf, h]
            psum5g = pp.tile([64, 64], F32)
            for grp in range(2):
                for hc in range(2):
                    nc.tensor.matmul(
                        psum5g[0:64, 32 * grp : 32 * grp + 32],
                        w5s[:, 128 * grp + 64 * hc : 128 * grp + 64 * hc + 64],
                        h4cols[:, 64 * hc + 32 * grp : 64 * hc + 32 * grp + 32],
                        start=(hc == 0),
                        stop=(hc == 1),
                    )
            # diag-extract via mask+reduce, then bias+relu -> predcol[(jl,o), g]
            ttr_dump = rp.tile([64, 64], F32)
            predcol = rp.tile([64, 2], F32)
            for grp in range(2):
                nc.vector.tensor_mul(
                    ttr_dump[:, 32 * grp : 32 * grp + 32],
                    psum5g[0:64, 32 * grp : 32 * grp + 32],
                    dmask,
                )
                nc.vector.tensor_reduce(
                    predcol[:, grp : grp + 1],
                    ttr_dump[:, 32 * grp : 32 * grp + 32],
                    mybir.AxisListType.X,
                    mybir.AluOpType.add,
                )
            nc.vector.tensor_add(predcol[:, :], predcol[:, :], b4bp)
            nc.vector.tensor_scalar_max(predcol[:, :], predcol[:, :], 0.0)

            # out[32g+jf] = sum_{jl,o} predcol[(jl,o), g] * msk2[(jl,o), 32g+jf]
            psum6 = pp.tile([1, J], F32)
            for grp in range(2):
                nc.tensor.matmul(
                    psum6[0:1, 32 * grp : 32 * grp + 32],
                    predcol[:, grp : grp + 1],
                    msk2[:, 32 * grp : 32 * grp + 32],
                    start=True,
                    stop=True,
                )
            yrow = rp.tile([1, J], F32)
            nc.vector.tensor_copy(yrow[0:1, :], psum6[0:1, :])
            nc.sync.dma_start(out, yrow[0:1, :])

    nc.compile()
    _CACHE["nc"] = nc
    return nc


def _bf(a):
    return np.ascontiguousarray(a.astype(NP_BF16))


def prep_core_inputs(inputs, c):
    """Host-side shard + layout prep for core c. inputs are np float32 arrays."""
    x = inputs["x"][0]  # [512]
    gi = np.arange(J * c, J * (c + 1))
    xg = x[gi]
    jj = np.arange(J)

    W1a_c = inputs["W1a"][gi]  # [64, 256, 512]
    w1t = _bf(W1a_c.transpose(2, 0, 1).reshape(512, J * H))

    # self-excite path entirely on host (tiny), folded into stage-4 bias
    dW2 = inputs["W2a"][gi, :, gi]  # [64, 2]
    h2 = np.maximum(dW2 * xg[:, None] + inputs["b2a"][gi], 0.0)
    r2 = np.maximum((inputs["W2b"][gi, 0] * h2).sum(-1) + inputs["b2b"][gi, 0], 0.0)

    # correction computed from the bf16-rounded operands (exact cancellation
    # of the diagonal term the device's bf16 matmul actually added)
    dW1 = W1a_c[jj, :, gi].astype(NP_BF16).astype(np.float32)  # [64, 256]
    xg_bf = xg.astype(NP_BF16).astype(np.float32)
    corr_jh = dW1 * xg_bf[:, None]
    corrt = corr_jh.reshape(J, 2, 128).transpose(2, 0, 1).reshape(128, 128)

    W4a_c = inputs["W4a"][gi]  # [64, 256, 65]
    bias4_jh = inputs["b4a"][gi] + W4a_c[:, :, 64] * r2[:, None]
    # h4 col layout = hc*64 + j
    bias4t = bias4_jh.reshape(J, 2, 128).transpose(2, 1, 0).reshape(128, 128)

    aux1 = np.ascontiguousarray(
        np.concatenate([corrt, bias4t], axis=1), dtype=np.float32
    )
    xbf = _bf(x.reshape(4, 128).T)  # [128, 4]

    # aux3: dmask | msk2 | b4bp   (stage-5 diag-extract + final combine)
    noise_c = inputs["noise"][gi]
    dmask = np.repeat(np.eye(32, dtype=np.float32), 2, axis=0)  # [64, 32]
    msk2 = np.zeros((64, 64), np.float32)
    for grp in range(2):
        jf = np.arange(32)
        msk2[2 * jf, 32 * grp + jf] = 1.0
        msk2[2 * jf + 1, 32 * grp + jf] = noise_c[32 * grp + jf]
    b4bp = (
        inputs["b4b"][gi].reshape(2, 32, 2).transpose(1, 2, 0).reshape(64, 2)
    )  # [2jl+o, grp]
    aux3 = np.ascontiguousarray(
        np.concatenate([dmask, msk2, b4bp], axis=1), dtype=np.float32
    )

    W1b_c = inputs["W1b"][gi]  # [64, 64, 256]
    w2t = _bf(
        W1b_c.reshape(J, 64, 2, 128).transpose(3, 0, 2, 1).reshape(128, J * 2 * 64)
    )

    # stage 4: branch pairs stacked on partitions (even j -> rows 0..63)
    W4m = W4a_c[:, :, 0:64]  # [j, h, m]
    T4 = W4m.reshape(J // 2, 2, H, 64)  # [u, par, h, m]
    w4t = _bf(T4.transpose(1, 3, 0, 2).reshape(128, (J // 2) * H))

    # w5p[hl, grp*128 + hc*64 + 2jl+o] = W4b[g(32grp+jl), o, hc*128+hl]
    W4b_c = inputs["W4b"][gi]  # [64, 2, 256]
    w5t = _bf(
        W4b_c.reshape(2, 32, 2, 2, 128).transpose(4, 0, 3, 1, 2).reshape(128, J * 4)
    )

    return {
        "w1t": w1t, "w2t": w2t, "w4t": w4t, "w5t": w5t,
        "xbf": xbf, "aux1": aux1, "aux3": aux3,
    }


def run(inputs, trace=False, **kw):
    inputs = {k: np.asarray(v, dtype=np.float32) for k, v in inputs.items()}
    nc = _build_nc()
    in_maps = [prep_core_inputs(inputs, c) for c in range(NCORES)]
    res = run_bass_kernel_spmd(
        nc, in_maps, core_ids=list(range(NCORES)), trace=trace, **kw
    )
    out = np.concatenate([res.results[c]["out"] for c in range(NCORES)], axis=1)
    return out.astype(np.float32), res


def kernel(**inputs):
    out, _ = run(inputs)
    return out


# revision 15
# speedup vs baseline: 3.7650x; 1.0101x over previous
"""Trainium2 Bass kernel for CausalTrajectoryPrediction (batched per-branch MLPs).

Math (per branch n of N=512, H=256, M=64):
    x_off = x with element n zeroed ; x_diag = only element n kept
    h1 = relu(W1a[n] @ x_off)            # [H]
    r1 = relu(W1b[n] @ h1)               # [M]
    r2 = relu(W2b[n] @ relu(W2a[n] @ x_diag + b2a[n]) + b2b[n])   # [1]
    h4 = relu(W4a[n] @ [r1; r2] + b4a[n])                          # [H]
    pred = relu(W4b[n] @ h4 + b4b[n])                              # [2]
    out[n] = pred[0] + noise[n] * pred[1]

Device strategy (8 cores, 64 branches each, expert-parallel):
  * W1a[n] @ x_off == W1a[n] @ x - W1a[n,:,n]*x[n]; the shared matvec is done
    on-device (weights stationary, x moving), the diagonal correction is a
    tiny host-side gather shipped as a [128,128] f32 tile (computed from the
    bf16-rounded operands so the subtraction cancels exactly).
  * The self-excite path (r2) only touches W2a's diagonal -> computed on host
    (512 branches x ~8 flops) and folded into an effective stage-4 bias:
    bias4_eff = b4a + W4a[:,:,64] * r2.  Stage 4 then contracts over m=0..63.
  * Weights + moving activations in bf16 (PSUM accumulation f32, all bias /
    correction math f32): f32 matmuls lower to 2x LDWEIGHTS+MATMUL passes on
    TRN2, and bf16 stationaries get fast-weight-load; bf16 also halves the
    32MB weight stream.
  * All weights are pre-transposed on host into [K-partition, free] layouts so
    each per-branch matvec is a single matmul with the activation vector as
    the moving operand; outputs land on PSUM partitions in exactly the layout
    the next stage consumes.
  * DMAs all issue on the sync (HWDGE/SP) ring -> FIFO in program order:
    the W1a stream first (paces stage-1 PE), then stage-2/4 weights arrive
    just-in-time for the tail stages.
"""

import ml_dtypes
import numpy as np

import concourse.bacc as bacc
import concourse.bass as bass
import concourse.mybir as mybir
import concourse.tile as tile
from concourse.bass_utils import run_bass_kernel_spmd

F32 = mybir.dt.float32
BF16 = mybir.dt.float16
NP_BF16 = np.float16
NCORES = 8
N, H, M = 512, 256, 64
J = N // NCORES  # 64 branches per core

_CACHE = {}


def _build_nc():
    if "nc" in _CACHE:
        return _CACHE["nc"]

    nc = bacc.Bacc(
        "TRN2", target_bir_lowering=False, debug=False, enable_asserts=False,
        num_devices=NCORES,
    )

    # --- DRAM I/O (per-core shapes) ---
    # w1t[i, j*256+h]              = W1a[g, h, i]           (g = 64*core + j)
    # w2t[hl, j*128+hc*64+m]       = W1b[g, m, hc*128+hl]
    # w4t[64*(j%2)+m, (j//2)*256+h]= W4a[g, h, m]   (m<64; col 64 folded in bias)
    # w5t[hl, j*4+hc*2+o]          = W4b[g, o, hc*128+hl]
    # xbf[p, ic] = x[128*ic+p]                     (bf16 moving operand)
    # aux1 = [corr(128) | bias4(128)]   -> [128, 256] f32
    #   corr[p, 2j+hh]  = bf16(W1a[g, hh*128+p, g]) * bf16(x[g])
    #   bias4[p, 2j+hh] = b4a[g, hh*128+p] + W4a[g, hh*128+p, 64]*r2_host[g]
    # aux2 = [b4bt(64) | noise2(64) | ones(1)]     -> [2, 129] f32
    w1t = nc.dram_tensor("w1t", [512, J * H], BF16, kind="ExternalInput").ap()
    w2t = nc.dram_tensor("w2t", [128, J * 2 * 64], BF16, kind="ExternalInput").ap()
    w4t = nc.dram_tensor("w4t", [128, (J // 2) * H], BF16, kind="ExternalInput").ap()
    w5t = nc.dram_tensor("w5t", [128, J * 4], BF16, kind="ExternalInput").ap()
    xbf = nc.dram_tensor("xbf", [128, 4], BF16, kind="ExternalInput").ap()
    aux1 = nc.dram_tensor("aux1", [128, 256], F32, kind="ExternalInput").ap()
    # aux3 = [dmask(32) | msk2(64) | b4bp(2)] -> [64, 98] f32
    #   dmask[2jl+o, jf] = (jl==jf)
    #   msk2[2jl+o, 32g+jf] = (jl==jf) * (1 if o==0 else noise[64c+32g+jf])
    #   b4bp[2jl+o, g] = b4b[64c+32g+jl, o]
    aux3 = nc.dram_tensor("aux3", [64, 98], F32, kind="ExternalInput").ap()
    out = nc.dram_tensor("out", [1, J], F32, kind="ExternalOutput").ap()

    # stage-1 free-dim chunk plan: small first chunk (fast rampup), 1 MiB rest
    CHUNK_COLS = [1024, 3072, 4096, 4096, 4096]

    with tile.TileContext(nc) as tc:
        with (
            tc.tile_pool(name="stream", bufs=12) as sp,
            tc.tile_pool(name="res", bufs=1) as rp,
            tc.tile_pool(name="psum", bufs=1, space=bass.MemorySpace.PSUM) as pp,
        ):
            # small resident tensors on the scalar (ACT) HWDGE ring so they
            # don't delay the w1t stream on the sync ring
            xbf_t = rp.tile([128, 4], BF16)
            nc.scalar.dma_start(xbf_t[:, :], xbf)
            aux1_t = rp.tile([128, 256], F32)
            nc.scalar.dma_start(aux1_t[:, :], aux1)
            aux3_t = rp.tile([64, 98], F32)
            nc.scalar.dma_start(aux3_t[:, :], aux3)
            w5s = rp.tile([128, J * 4], BF16)
            nc.scalar.dma_start(w5s[:, :], w5t)

            corrt = aux1_t[:, 0:128]
            bias4t = aux1_t[:, 128:256]
            dmask = aux3_t[:, 0:32]
            msk2 = aux3_t[:, 32:96]
            b4bp = aux3_t[:, 96:98]

            w2s = rp.tile([128, J * 2 * 64], BF16)
            w4s = rp.tile([128, (J // 2) * H], BF16)

            # y1 split across two PSUM banks so each half's relu can run while
            # stage-1 matmuls still write the other bank
            y1a = pp.tile([128, 64], F32)
            y1b = pp.tile([128, 64], F32)
            h1f = rp.tile([128, 128], F32)
            h1sb = rp.tile([128, 128], BF16)

            def relu_half(half):
                lo = 64 * half
                y1h = y1a if half == 0 else y1b
                nc.vector.tensor_sub(
                    h1f[:, lo : lo + 64], y1h[:, 0:64], corrt[:, lo : lo + 64]
                )
                nc.vector.tensor_scalar_max(
                    h1sb[:, lo : lo + 64], h1f[:, lo : lo + 64], 0.0
                )

            # ---- stage 1: y1[j,h] = sum_i W1a[g,h,i] x[i] ----
            # 128 psum columns t = 2j+hh ; 4 accumulating matmuls each (i-chunks)
            # weight stream alternates between the two HWDGE rings
            col0 = 0
            for ncols in CHUNK_COLS:
                tiles_u = []
                for ic in range(4):
                    wt = sp.tile([128, ncols], BF16, tag="w1s")
                    eng = nc.sync if ic % 2 == 0 else nc.scalar
                    eng.dma_start(
                        wt[:, :], w1t[128 * ic : 128 * (ic + 1), col0 : col0 + ncols]
                    )
                    tiles_u.append(wt)
                for tt in range(ncols // 128):
                    t = col0 // 128 + tt
                    y1h = y1a if t < 64 else y1b
                    for ic in range(4):
                        nc.tensor.matmul(
                            y1h[:, t % 64 : t % 64 + 1],
                            tiles_u[ic][:, 128 * tt : 128 * (tt + 1)],
                            xbf_t[:, ic : ic + 1],
                            start=(ic == 0),
                            stop=(ic == 3),
                        )
                col0 += ncols
                if col0 == 8192:
                    relu_half(0)  # first 64 cols complete -> relu bank A early

            # stage-2/4 weights stream on both rings right after w1t
            for k in range(4):
                eng = nc.sync if k % 2 == 0 else nc.scalar
                eng.dma_start(
                    w2s[:, 2048 * k : 2048 * (k + 1)], w2t[:, 2048 * k : 2048 * (k + 1)]
                )
            for k in range(4):
                eng = nc.sync if k % 2 == 1 else nc.scalar
                eng.dma_start(
                    w4s[:, 2048 * k : 2048 * (k + 1)], w4t[:, 2048 * k : 2048 * (k + 1)]
                )

            relu_half(1)

            # ---- stage 2: r1[j,m] = relu(sum_h W1b[g,m,h] h1[j,h]) ----
            # even j -> psum rows 0..63, odd j -> rows 64..127 (col-group tiling)
            psum2 = pp.tile([128, J], F32)
            nc.vector.memset(psum2[:, :], 0.0)
            for j in range(J):
                off = 64 * (j % 2)
                for hc in range(2):
                    nc.tensor.matmul(
                        psum2[off : off + 64, j : j + 1],
                        w2s[:, j * 128 + hc * 64 : j * 128 + hc * 64 + 64],
                        h1sb[:, 2 * j + hc : 2 * j + hc + 1],
                        start=(hc == 0),
                        stop=(hc == 1),
                    )
            r1cols = rp.tile([128, J], BF16)
            nc.vector.tensor_scalar_max(r1cols[:, :], psum2[:, :], 0.0)

            # ---- stage 4: h4[j,h] = relu(sum_m W4a[g,h,m] r1[j,m] + bias4eff) ----
            # one K=128 stationary per branch PAIR (the wrong-parity half of
            # each r1cols column is exactly zero), h4 col layout = hc*64 + j
            psum4 = pp.tile([128, 128], F32)
            for u_ in range(J // 2):
                for hc in range(2):
                    for par in range(2):
                        j = 2 * u_ + par
                        nc.tensor.matmul(
                            psum4[:, hc * 64 + j : hc * 64 + j + 1],
                            w4s[:, u_ * 256 + hc * 128 : u_ * 256 + hc * 128 + 128],
                            r1cols[:, j : j + 1],
                            start=True,
                            stop=True,
                        )
            h4f = rp.tile([128, 128], F32)
            nc.vector.tensor_add(h4f[:, :], psum4[:, :], bias4t)
            h4cols = rp.tile([128, 128], BF16)
            nc.vector.tensor_scalar_max(h4cols[:, :], h4f[:, :], 0.0)

            # ---- stage 5: 32 branches per stationary, F=32 moving ----
            # psum5g[(2jl+o), 32g+jf] = sum_h W4b[32g+jl, o, h] h4[32g+jf, h]
            psum5g = pp.tile([64, 64], F32)
            for grp in range(2):
                for hc in range(2):
                    nc.tensor.matmul(
                        psum5g[0:64, 32 * grp : 32 * grp + 32],
                        w5s[:, 128 * grp + 64 * hc : 128 * grp + 64 * hc + 64],
                        h4cols[:, 64 * hc + 32 * grp : 64 * hc + 32 * grp + 32],
                        start=(hc == 0),
                        stop=(hc == 1),
                    )
            # diag-extract via mask+reduce, then bias+relu -> predcol[(jl,o), g]
            ttr_dump = rp.tile([64, 64], F32)
            predcol = rp.tile([64, 2], F32)
            for grp in range(2):
                nc.vector.tensor_mul(
                    ttr_dump[:, 32 * grp : 32 * grp + 32],
                    psum5g[0:64, 32 * grp : 32 * grp + 32],
                    dmask,
                )
                nc.vector.tensor_reduce(
                    predcol[:, grp : grp + 1],
                    ttr_dump[:, 32 * grp : 32 * grp + 32],
                    mybir.AxisListType.X,
                    mybir.AluOpType.add,
                )
            nc.vector.tensor_add(predcol[:, :], predcol[:, :], b4bp)
            nc.vector.tensor_scalar_max(predcol[:, :], predcol[:, :], 0.0)

            # out[32g+jf] = sum_{jl,o} predcol[(jl,o), g] * msk2[(jl,o), 32g+jf]
            psum6 = pp.tile([1, J], F32)
            for grp in range(2):
                nc.tensor.matmul(
                    psum6[0:1, 32 * grp : 32 * grp + 32],
                    predcol[:, grp : grp + 1],
                    msk2[:, 32 * grp : 32 * grp + 32],
                    start=True,
                    stop=True,
                )
            yrow = rp.tile([1, J], F32)
            nc.vector.tensor_copy(yrow[0:1, :], psum6[0:1, :])
            nc.sync.dma_start(out, yrow[0:1, :])

    nc.compile()
    _CACHE["nc"] = nc
    return nc


def _bf(a):
    return np.ascontiguousarray(a.astype(NP_BF16))


def prep_core_inputs(inputs, c):
    """Host-side shard + layout prep for core c. inputs are np float32 arrays."""
    x = inputs["x"][0]  # [512]
    gi = np.arange(J * c, J * (c + 1))
    xg = x[gi]
    jj = np.arange(J)

    W1a_c = inputs["W1a"][gi]  # [64, 256, 512]
    w1t = _bf(W1a_c.transpose(2, 0, 1).reshape(512, J * H))

    # self-excite path entirely on host (tiny), folded into stage-4 bias
    dW2 = inputs["W2a"][gi, :, gi]  # [64, 2]
    h2 = np.maximum(dW2 * xg[:, None] + inputs["b2a"][gi], 0.0)
    r2 = np.maximum((inputs["W2b"][gi, 0] * h2).sum(-1) + inputs["b2b"][gi, 0], 0.0)

    # correction computed from the bf16-rounded operands (exact cancellation
    # of the diagonal term the device's bf16 matmul actually added)
    dW1 = W1a_c[jj, :, gi].astype(NP_BF16).astype(np.float32)  # [64, 256]
    xg_bf = xg.astype(NP_BF16).astype(np.float32)
    corr_jh = dW1 * xg_bf[:, None]
    corrt = corr_jh.reshape(J, 2, 128).transpose(2, 0, 1).reshape(128, 128)

    W4a_c = inputs["W4a"][gi]  # [64, 256, 65]
    bias4_jh = inputs["b4a"][gi] + W4a_c[:, :, 64] * r2[:, None]
    # h4 col layout = hc*64 + j
    bias4t = bias4_jh.reshape(J, 2, 128).transpose(2, 1, 0).reshape(128, 128)

    aux1 = np.ascontiguousarray(
        np.concatenate([corrt, bias4t], axis=1), dtype=np.float32
    )
    xbf = _bf(x.reshape(4, 128).T)  # [128, 4]

    # aux3: dmask | msk2 | b4bp   (stage-5 diag-extract + final combine)
    noise_c = inputs["noise"][gi]
    dmask = np.repeat(np.eye(32, dtype=np.float32), 2, axis=0)  # [64, 32]
    msk2 = np.zeros((64, 64), np.float32)
    for grp in range(2):
        jf = np.arange(32)
        msk2[2 * jf, 32 * grp + jf] = 1.0
        msk2[2 * jf + 1, 32 * grp + jf] = noise_c[32 * grp + jf]
    b4bp = (
        inputs["b4b"][gi].reshape(2, 32, 2).transpose(1, 2, 0).reshape(64, 2)
    )  # [2jl+o, grp]
    aux3 = np.ascontiguousarray(
        np.concatenate([dmask, msk2, b4bp], axis=1), dtype=np.float32
    )

    W1b_c = inputs["W1b"][gi]  # [64, 64, 256]
    w2t = _bf(
        W1b_c.reshape(J, 64, 2, 128).transpose(3, 0, 2, 1).reshape(128, J * 2 * 64)
    )

    # stage 4: branch pairs stacked on partitions (even j -> rows 0..63)
    W4m = W4a_c[:, :, 0:64]  # [j, h, m]
    T4 = W4m.reshape(J // 2, 2, H, 64)  # [u, par, h, m]
    w4t = _bf(T4.transpose(1, 3, 0, 2).reshape(128, (J // 2) * H))

    # w5p[hl, grp*128 + hc*64 + 2jl+o] = W4b[g(32grp+jl), o, hc*128+hl]
    W4b_c = inputs["W4b"][gi]  # [64, 2, 256]
    w5t = _bf(
        W4b_c.reshape(2, 32, 2, 2, 128).transpose(4, 0, 3, 1, 2).reshape(128, J * 4)
    )

    return {
        "w1t": w1t, "w2t": w2t, "w4t": w4t, "w5t": w5t,
        "xbf": xbf, "aux1": aux1, "aux3": aux3,
    }


def run(inputs, trace=False, **kw):
    inputs = {k: np.asarray(v, dtype=np.float32) for k, v in inputs.items()}
    nc = _build_nc()
    in_maps = [prep_core_inputs(inputs, c) for c in range(NCORES)]
    res = run_bass_kernel_spmd(
        nc, in_maps, core_ids=list(range(NCORES)), trace=trace, **kw
    )
    out = np.concatenate([res.results[c]["out"] for c in range(NCORES)], axis=1)
    return out.astype(np.float32), res


def kernel(**inputs):
    out, _ = run(inputs)
    return out
